# revision 1
# baseline (speedup 1.0000x reference)
"""DNANet Bass kernel v2: bf16 tables, no q-gather, dual SWDGE queues.

See kernel_lib.py docstring for the overall scheme. v2 changes:
- k/v tables, gathered data, onehot, messages, AllGather in bf16
  (accumulation and softmax stay fp32).
- Per-edge queries come from a per-tile onehot-transpose matmul on the
  TensorEngine instead of a dma_gather (halves Q7 descriptor-gen work).
- kv gathers alternate between two SWDGE queues.
"""
import math
import numpy as np
import concourse.bacc as bacc
import concourse.mybir as mybir
import concourse.tile as tile
from concourse.masks import make_identity

F32, I32, I16 = mybir.dt.float32, mybir.dt.int32, mybir.dt.int16
BF16 = mybir.dt.bfloat16
HID, H, DH, INC, OUTC = 64, 4, 16, 128, 16
LMAX = 3
KVROW = LMAX * 2 * HID  # 384 elements per node row in kv table
HALF = 25088            # row split so int16 indices stay positive
C_MAX = 8               # chunks per superchunk (<=1024 idx per dma_gather)


def _wrap16(arr_cm):
    """[tot_ch, 128] int -> [128, tot_ch*8] int16 ucode idx layout.

    Replicated into all eight 16-partition groups so any SWDGE queue's
    Q7 core pair can read its copy."""
    tc_, _ = arr_cm.shape
    a = arr_cm.reshape(tc_, 8, 16).transpose(2, 0, 1).reshape(16, tc_ * 8)
    out = np.zeros((128, tc_ * 8), np.int16)
    for r in range(8):
        out[r * 16:(r + 1) * 16] = a
    return out


def make_cfg(n_nodes, edge_index, n_cores=8):
    tpc = math.ceil(n_nodes / n_cores / 128)
    npad = n_cores * tpc * 128
    nsh = tpc * 128
    half = HALF if npad + 128 > 32000 else npad + 128

    src = np.asarray(edge_index[0], dtype=np.int64)
    dst = np.asarray(edge_index[1], dtype=np.int64)
    loop = np.arange(n_nodes, dtype=np.int64)
    src = np.concatenate([src, loop])
    dst = np.concatenate([dst, loop])

    deg = np.bincount(dst, minlength=npad).astype(np.float32)
    dinv = np.where(deg > 0, 1.0 / np.sqrt(np.maximum(deg, 1.0)), 0.0).astype(np.float32)

    key = dst * 2 + (src >= half)
    order = np.argsort(key, kind="stable")
    src_s, dst_s = src[order], dst[order]

    cnt_lo = np.zeros((n_cores, tpc), np.int64)
    cnt_hi = np.zeros((n_cores, tpc), np.int64)
    core_of = dst_s // nsh
    ltile = (dst_s % nsh) // 128
    is_hi = src_s >= half
    np.add.at(cnt_lo, (core_of[~is_hi], ltile[~is_hi]), 1)
    np.add.at(cnt_hi, (core_of[is_hi], ltile[is_hi]), 1)
    kt_lo = np.maximum(np.ceil(cnt_lo.max(axis=0) / 128).astype(np.int64), 1)
    kt_hi = np.ceil(cnt_hi.max(axis=0) / 128).astype(np.int64)
    kt = kt_lo + kt_hi
    tot_ch = int(kt.sum())
    chunk_base = np.concatenate([[0], np.cumsum(kt)]).astype(np.int64)

    kvidx_cm = np.zeros((n_cores, tot_ch, 128), np.int64)
    dloc_cm = np.full((n_cores, tot_ch, 128), 128, np.int64)
    for c in range(n_cores):
        for t in range(tpc):
            g0 = c * nsh + t * 128
            lo = np.searchsorted(dst_s, g0)
            hi = np.searchsorted(dst_s, g0 + 128)
            if hi == lo:
                continue
            sseg = src_s[lo:hi]
            dseg = dst_s[lo:hi]
            hseg = sseg >= half
            for half_i, mask, base_ch in (
                (0, ~hseg, chunk_base[t]),
                (1, hseg, chunk_base[t] + kt_lo[t]),
            ):
                sv = sseg[mask]
                dv = dseg[mask]
                n = len(sv)
                if n == 0:
                    continue
                ch = base_ch + np.arange(n) // 128
                lane = np.arange(n) % 128
                kvidx_cm[c, ch, lane] = sv - (half if half_i else 0)
                dloc_cm[c, ch, lane] = dv - g0

    dinvT = dinv.reshape(-1, 128).T.copy()  # [128, nt_g]
    return dict(
        n_cores=n_cores, tpc=tpc, npad=npad, nsh=nsh, n_nodes=n_nodes,
        half=half,
        kt=[int(k) for k in kt], kt_lo=[int(k) for k in kt_lo],
        kt_hi=[int(k) for k in kt_hi],
        tot_ch=tot_ch, chunk_base=[int(b) for b in chunk_base],
        kvidx=np.stack([_wrap16(kvidx_cm[c].astype(np.int16)) for c in range(n_cores)]),
        dlocT=np.ascontiguousarray(
            dloc_cm.astype(np.int32).transpose(0, 2, 1)),  # [c, 128, tot_ch]
        dinvT=dinvT,
    )


def prep_inputs(cfg, x, W_lin, b_lin, Wq, bq, Wk, bk, Wv, bv, W_out, b_out):
    ncore, npad, tpc = cfg["n_cores"], cfg["npad"], cfg["tpc"]
    n = x.shape[0]
    xT = np.zeros((INC, npad), np.float32)
    xT[:, :n] = np.asarray(x, np.float32).T
    iota = np.broadcast_to(np.arange(128, dtype=np.int32), (128, 128)).copy()
    base = {
        "xT": xT,
        "wlin": np.asarray(W_lin, np.float32),
        "wq": np.asarray(Wq, np.float32),
        "wk": np.asarray(Wk, np.float32),
        "wv": np.asarray(Wv, np.float32),
        "wout": np.asarray(W_out, np.float32),
        "blin_col": np.asarray(b_lin, np.float32).reshape(HID, 1),
        "bq_r": np.asarray(bq, np.float32).reshape(1, HID),
        "bk_rep": np.tile(np.asarray(bk, np.float32).reshape(1, HID), (1, 4)),
        "bv_rep": np.tile(np.asarray(bv, np.float32).reshape(1, HID), (1, 4)),
        "bout_r": np.asarray(b_out, np.float32).reshape(1, OUTC),
        "iota_i": iota,
        "dinvT": cfg["dinvT"],
    }
    in_maps = []
    for c in range(ncore):
        m = dict(base)
        m["dinvL"] = np.ascontiguousarray(cfg["dinvT"][:, c * tpc:(c + 1) * tpc])
        m["kvidx"] = cfg["kvidx"][c]
        m["dlocT"] = cfg["dlocT"][c]
        in_maps.append(m)
    return in_maps


def build_kernel(cfg):
    ncore, tpc, npad, nsh = cfg["n_cores"], cfg["tpc"], cfg["npad"], cfg["nsh"]
    kt, kt_lo, kt_hi = cfg["kt"], cfg["kt_lo"], cfg["kt_hi"]
    tot_ch, chunk_base, half = cfg["tot_ch"], cfg["chunk_base"], cfg["half"]
    nt_g = ncore * tpc
    ng = nt_g // 4

    nc = bacc.Bacc("TRN2", target_bir_lowering=False, debug=False,
                   num_devices=ncore, num_swdge_queues=2)

    xT = nc.dram_tensor("xT", [INC, npad], F32, kind="ExternalInput")
    wlin = nc.dram_tensor("wlin", [INC, HID], F32, kind="ExternalInput")
    wq = nc.dram_tensor("wq", [HID, HID], F32, kind="ExternalInput")
    wk = nc.dram_tensor("wk", [HID, HID], F32, kind="ExternalInput")
    wv = nc.dram_tensor("wv", [HID, HID], F32, kind="ExternalInput")
    wout = nc.dram_tensor("wout", [HID, OUTC], F32, kind="ExternalInput")
    blin_col = nc.dram_tensor("blin_col", [HID, 1], F32, kind="ExternalInput")
    bq_r = nc.dram_tensor("bq_r", [1, HID], F32, kind="ExternalInput")
    bk_rep = nc.dram_tensor("bk_rep", [1, 4 * HID], F32, kind="ExternalInput")
    bv_rep = nc.dram_tensor("bv_rep", [1, 4 * HID], F32, kind="ExternalInput")
    bout_r = nc.dram_tensor("bout_r", [1, OUTC], F32, kind="ExternalInput")
    iota_i = nc.dram_tensor("iota_i", [128, 128], I32, kind="ExternalInput")
    dinvT_d = nc.dram_tensor("dinvT", [128, nt_g], F32, kind="ExternalInput")
    dinvL_d = nc.dram_tensor("dinvL", [128, tpc], F32, kind="ExternalInput")
    kvidx_d = nc.dram_tensor("kvidx", [128, tot_ch * 8], I16, kind="ExternalInput")
    dlocT_d = nc.dram_tensor("dlocT", [128, tot_ch], I32, kind="ExternalInput")
    y = nc.dram_tensor("y", [nsh, OUTC], F32, kind="ExternalOutput")
    debug = cfg.get("debug", False)
    if debug:
        dbg_out1 = nc.dram_tensor("dbg_out1", [nsh, HID], F32, kind="ExternalOutput")
        dbg_out2 = nc.dram_tensor("dbg_out2", [nsh, HID], F32, kind="ExternalOutput")

    with tile.TileContext(nc) as tc:
        import contextlib
        ctx = contextlib.ExitStack()
        with ctx:
            cpool = ctx.enter_context(tc.tile_pool(name="const", bufs=1))
            dram = ctx.enter_context(tc.tile_pool(name="dram", bufs=1, space="DRAM"))

            kvtab = dram.tile([npad + 128, KVROW], BF16, name="kvtab")
            qtabs = [dram.tile([nsh + 128, HID], BF16, name=f"qtab{i}") for i in (0, 1)]
            agin = [dram.tile([nsh, HID], BF16, name=f"agin{s}") for s in (1, 2)]
            agout = [dram.tile([npad, HID], BF16, name=f"agout{s}", addr_space="Shared")
                     for s in (1, 2)]

            def load_const(dt_, shape, src_ap, name):
                t_ = cpool.tile(shape, dt_, name=name)
                nc.sync.dma_start(t_[:], src_ap)
                return t_

            wlin_s = load_const(F32, [INC, HID], wlin[:], "wlin_s")
            wq_s = load_const(F32, [HID, HID], wq[:], "wq_s")
            wk_s = load_const(F32, [HID, HID], wk[:], "wk_s")
            wv_s = load_const(F32, [HID, HID], wv[:], "wv_s")
            wout_s = load_const(F32, [HID, OUTC], wout[:], "wout_s")
            blin_s = load_const(F32, [HID, 1], blin_col[:], "blin_s")
            bq_s = load_const(F32, [1, HID], bq_r[:], "bq_s")
            bk_s = load_const(F32, [1, 4 * HID], bk_rep[:], "bk_s")
            bv_s = load_const(F32, [1, 4 * HID], bv_rep[:], "bv_s")
            bout_s = load_const(F32, [1, OUTC], bout_r[:], "bout_s")
            iota_s = load_const(I32, [128, 128], iota_i[:], "iota_s")
            dinvT_s = load_const(F32, [128, nt_g], dinvT_d[:], "dinvT_s")
            dinvL_s = load_const(F32, [128, tpc], dinvL_d[:], "dinvL_s")
            iden = cpool.tile([128, 128], F32, name="iden")
            make_identity(nc, iden[:])
            iden_bf = cpool.tile([128, 128], BF16, name="iden_bf")
            nc.vector.tensor_copy(iden_bf[:], iden[:])
            wk_bf = cpool.tile([HID, HID], BF16, name="wk_bf")
            nc.vector.tensor_copy(wk_bf[:], wk_s[:])
            wv_bf = cpool.tile([HID, HID], BF16, name="wv_bf")
            nc.vector.tensor_copy(wv_bf[:], wv_s[:])
            ones_r = cpool.tile([1, 128], F32, name="ones_r")
            nc.vector.memset(ones_r[:], 1.0)

            sb_xt = ctx.enter_context(tc.tile_pool(name="sb_xt", bufs=3))
            sb_ht = ctx.enter_context(tc.tile_pool(name="sb_ht", bufs=3))
            sb_kv = ctx.enter_context(tc.tile_pool(name="sb_kv", bufs=3))
            sb_q = ctx.enter_context(tc.tile_pool(name="sb_q", bufs=3))
            sb_idx = ctx.enter_context(tc.tile_pool(name="sb_idx", bufs=6))
            sb_oh = ctx.enter_context(tc.tile_pool(name="sb_oh", bufs=3))
            sb_g = ctx.enter_context(tc.tile_pool(name="sb_g", bufs=3))
            sb_ve = ctx.enter_context(tc.tile_pool(name="sb_ve", bufs=3))
            sb_sm = ctx.enter_context(tc.tile_pool(name="sb_sm", bufs=3))
            sb_out = ctx.enter_context(tc.tile_pool(name="sb_out", bufs=3))
            ps_big = ctx.enter_context(tc.tile_pool(name="ps_big", bufs=1, space="PSUM"))
            ps_kv = ctx.enter_context(tc.tile_pool(name="ps_kv", bufs=1, space="PSUM"))
            ps_out = ctx.enter_context(tc.tile_pool(name="ps_out", bufs=2, space="PSUM"))
            ps_qe = ctx.enter_context(tc.tile_pool(name="ps_qe", bufs=2, space="PSUM"))
            ps_oht = ctx.enter_context(tc.tile_pool(name="ps_oht", bufs=1, space="PSUM"))

            AF, ALU = mybir.ActivationFunctionType, mybir.AluOpType
            AX = mybir.AxisListType
            qctr = [0]

            def next_q():
                qctr[0] += 1
                return qctr[0] % 2

            # ================= table slice build =================
            def build_slice(s):
                for g in range(ng):
                    if s == 0:
                        xt_t = sb_xt.tile([INC, 512], F32, name="xt_t")
                        nc.sync.dma_start(xt_t[:], xT[:, g * 512:(g + 1) * 512])
                        htp = ps_big.tile([HID, 512], F32, name="htp", space="PSUM")
                        for b in range(4):
                            nc.tensor.matmul(htp[:, b * 128:(b + 1) * 128], lhsT=wlin_s[:],
                                             rhs=xt_t[:, b * 128:(b + 1) * 128],
                                             start=(b == 0), stop=(b == 3))
                        hts = sb_ht.tile([HID, 512], F32, name="hts")
                        nc.vector.tensor_scalar(out=hts[:], in0=htp[:], scalar1=blin_s[:],
                                                scalar2=None, op0=ALU.add)
                        wkx, wvx = wk_s, wv_s
                    else:
                        xs_nm = sb_xt.tile([128, 4 * HID], BF16, name="xs_nm")
                        nc.sync.dma_start(
                            xs_nm[:].rearrange("p (b d) -> p b d", b=4, d=HID),
                            agout[s - 1][g * 512:(g + 1) * 512, :]
                            .rearrange("(b p) d -> p b d", p=128))
                        htp = ps_big.tile([HID, 512], BF16, name="htp", space="PSUM", padded_shape=[HID, 1024])
                        for b in range(4):
                            nc.tensor.transpose(htp[:, b * 128:(b + 1) * 128],
                                                in_=xs_nm[:, b * HID:(b + 1) * HID],
                                                identity=iden_bf[:])
                        hts = sb_ht.tile([HID, 512], BF16, name="hts", padded_shape=[HID, 1024])
                        nc.vector.tensor_copy(hts[:], htp[:])
                        wkx, wvx = wk_bf, wv_bf

                    kp = ps_kv.tile([128, 4 * HID], F32, name="kp", space="PSUM")
                    vp = ps_kv.tile([128, 4 * HID], F32, name="vp", space="PSUM")
                    for b in range(4):
                        nc.tensor.matmul(kp[:, b * HID:(b + 1) * HID],
                                         lhsT=hts[:, b * 128:(b + 1) * 128], rhs=wkx[:],
                                         start=(b == 0), stop=False)
                    nc.tensor.matmul(kp[:], lhsT=ones_r[:], rhs=bk_s[:],
                                     start=False, stop=True)
                    for b in range(4):
                        nc.tensor.matmul(vp[:, b * HID:(b + 1) * HID],
                                         lhsT=hts[:, b * 128:(b + 1) * 128], rhs=wvx[:],
                                         start=(b == 0), stop=False)
                    nc.tensor.matmul(vp[:], lhsT=ones_r[:], rhs=bv_s[:],
                                     start=False, stop=True)

                    kvsb = sb_kv.tile([128, 4 * 128], BF16, name="kvsb")
                    kvv = kvsb[:].rearrange("p (b s d) -> p b s d", b=4, s=2, d=HID)
                    nc.vector.tensor_copy(kvv[:, :, 0, :],
                                          kp[:].rearrange("p (b d) -> p b d", b=4, d=HID))
                    nc.vector.tensor_tensor(
                        kvv[:, :, 1, :],
                        vp[:].rearrange("p (b d) -> p b d", b=4, d=HID),
                        dinvT_s[:, g * 4:(g + 1) * 4]
                        .rearrange("p (b u) -> p b u", b=4, u=1)
                        .to_broadcast([128, 4, HID]),
                        ALU.mult)
                    nc.sync.dma_start(
                        kvtab[g * 512:(g + 1) * 512, s * 128:(s + 1) * 128]
                        .rearrange("(b p) d -> p b d", p=128),
                        kvsb[:].rearrange("p (b d) -> p b d", b=4, d=128))

            def build_q_from_out(outsb, t, qi):
                htp = ps_big.tile([HID, 512], F32, name="htp", space="PSUM")
                nc.tensor.transpose(htp[:, :128], in_=outsb[:], identity=iden[:])
                hts = sb_ht.tile([HID, 512], F32, name="hts")
                nc.vector.tensor_copy(hts[:, :128], htp[:, :128])
                qp = ps_kv.tile([128, 4 * HID], F32, name="kp", space="PSUM")
                nc.tensor.matmul(qp[:, :HID], lhsT=hts[:, :128], rhs=wq_s[:],
                                 start=True, stop=False)
                nc.tensor.matmul(qp[:, :HID], lhsT=ones_r[:], rhs=bq_s[:],
                                 start=False, stop=True)
                qsb = sb_q.tile([128, HID], BF16, name="qsb")
                nc.vector.tensor_copy(qsb[:], qp[:, :HID])
                nc.sync.dma_start(qtabs[qi][t * 128:(t + 1) * 128, :], qsb[:])

            # ================= edge pass =================
            def edge_layer(ell, agidx):
                L = ell
                for t in range(tpc):
                    po = ps_out.tile([128, HID], F32, name="po", space="PSUM")
                    if ell > 1:
                        qtile = sb_q.tile([128, HID], BF16, name="qtile")
                        nc.sync.dma_start(qtile[:], qtabs[ell - 2][t * 128:(t + 1) * 128, :])
                    n_ch = kt[t]
                    base = chunk_base[t]
                    scs = []
                    for seg_o, seg_n, rb in ((0, kt_lo[t], 0), (kt_lo[t], kt_hi[t], half)):
                        o = 0
                        while o < seg_n:
                            w = min(C_MAX, seg_n - o)
                            scs.append((seg_o + o, w, rb))
                            o += w
                    done = 0
                    for (o, cc, rb) in scs:
                        cb = base + o
                        dli = sb_idx.tile([128, C_MAX], I32, name="dli")
                        nc.sync.dma_start(dli[:, :cc], dlocT_d[:, cb:cb + cc])
                        kvi = sb_idx.tile([128, C_MAX * 8], I16, name="kvi")
                        nc.sync.dma_start(kvi[:, :cc * 8], kvidx_d[:, cb * 8:(cb + cc) * 8])

                        oh = sb_oh.tile([128, C_MAX * 128], BF16, name="oh")
                        nc.vector.tensor_tensor(
                            oh[:, :cc * 128].rearrange("p (c i) -> p c i", c=cc, i=128),
                            dli[:, :cc].rearrange("p (c u) -> p c u", c=cc, u=1)
                            .to_broadcast([128, cc, 128]),
                            iota_s[:].rearrange("p (u i) -> p u i", u=1, i=128)
                            .to_broadcast([128, cc, 128]),
                            ALU.is_equal)

                        if ell == 1:
                            vg = sb_g.tile([128, C_MAX * 128], BF16, name="vg1")
                            nc.gpsimd.dma_gather(
                                out_ap=vg[:, :cc * 128].rearrange(
                                    "p (n d) -> p n d", d=128),
                                in_ap=kvtab[rb:, 0:128],
                                idxs_ap=kvi[:, :cc * 8],
                                num_idxs=cc * 128, num_idxs_reg=cc * 128,
                                elem_size=128, elem_step=KVROW,
                                queue_num=next_q())
                            vgv = vg[:, :cc * 128].rearrange("p (c s d) -> p c s d",
                                                             s=2, d=HID)
                            msg, mslice = vg, lambda k: vgv[:, k, 1, :]
                        else:
                            kvg = sb_g.tile([128, C_MAX * LMAX * 128], BF16, name="kvg")
                            nc.gpsimd.dma_gather(
                                out_ap=kvg[:, :cc * L * 128].rearrange(
                                    "p (n d) -> p n d", d=L * 128),
                                in_ap=kvtab[rb:, :L * 128],
                                idxs_ap=kvi[:, :cc * 8],
                                num_idxs=cc * 128, num_idxs_reg=cc * 128,
                                elem_size=L * 128, elem_step=KVROW,
                                queue_num=next_q())

                            qe = ps_qe.tile([128, C_MAX * HID], F32, name="qe",
                                            space="PSUM")
                            for k in range(cc):
                                ohtp = ps_oht.tile([128, 128], BF16, name="ohtp",
                                                   space="PSUM")
                                nc.tensor.transpose(ohtp[:],
                                                    in_=oh[:, k * 128:(k + 1) * 128],
                                                    identity=iden_bf[:])
                                ohts = sb_oh.tile([128, 128], BF16, name="ohts")
                                nc.vector.tensor_copy(ohts[:], ohtp[:])
                                nc.tensor.matmul(qe[:, k * HID:(k + 1) * HID],
                                                 lhsT=ohts[:], rhs=qtile[:],
                                                 start=True, stop=True)

                            kvv = kvg[:, :cc * L * 128].rearrange(
                                "p (c l s h d) -> p c l s h d", c=cc, l=L, s=2, h=H, d=DH)
                            qv = qe[:, :cc * HID].rearrange("p (c h d) -> p c h d",
                                                            h=H, d=DH)
                            satt = sb_sm.tile([128, C_MAX * H * LMAX], F32, name="satt")
                            sv = satt[:, :cc * H * L].rearrange("p (c h l) -> p c h l",
                                                                h=H, l=L)
                            qk = sb_ve.tile([128, C_MAX * HID], F32, name="qk")
                            qkv = qk[:, :cc * HID].rearrange("p (c h d) -> p c h d",
                                                             h=H, d=DH)
                            for l in range(L):
                                nc.vector.tensor_tensor(qkv, qv, kvv[:, :, l, 0, :, :],
                                                        ALU.mult)
                                nc.vector.reduce_sum(sv[:, :, :, l], qkv, axis=AX.X)
                            eatt = sb_sm.tile([128, C_MAX * H * LMAX], F32, name="eatt")
                            nc.scalar.activation(eatt[:, :cc * H * L], satt[:, :cc * H * L],
                                                 AF.Exp, scale=1.0 / math.sqrt(DH))
                            den = sb_sm.tile([128, C_MAX * H], F32, name="den")
                            nc.vector.reduce_sum(
                                den[:, :cc * H].rearrange("p (c h) -> p c h", h=H),
                                eatt[:, :cc * H * L].rearrange("p (c h l) -> p c h l",
                                                               h=H, l=L),
                                axis=AX.X)
                            rden = sb_sm.tile([128, C_MAX * H], F32, name="rden")
                            nc.vector.reciprocal(rden[:, :cc * H], den[:, :cc * H])
                            att = sb_sm.tile([128, C_MAX * H * LMAX], F32, name="att")
                            av = att[:, :cc * H * L].rearrange("p (c h l) -> p c h l",
                                                               h=H, l=L)
                            nc.vector.tensor_tensor(
                                av,
                                eatt[:, :cc * H * L].rearrange("p (c h l) -> p c h l",
                                                               h=H, l=L),
                                rden[:, :cc * H].rearrange("p (c h u) -> p c h u",
                                                           h=H, u=1)
                                .to_broadcast([128, cc, H, L]),
                                ALU.mult)
                            msg = sb_ve.tile([128, C_MAX * HID], BF16, name="msg")
                            wvt = sb_ve.tile([128, C_MAX * HID], BF16, name="wvt")
                            mv = msg[:, :cc * HID].rearrange("p (c h d) -> p c h d",
                                                             h=H, d=DH)
                            wvv = wvt[:, :cc * HID].rearrange("p (c h d) -> p c h d",
                                                              h=H, d=DH)
                            for l in range(L):
                                nc.vector.tensor_tensor(
                                    mv if l == 0 else wvv,
                                    av[:, :, :, l].to_broadcast([128, cc, H, DH]),
                                    kvv[:, :, l, 1, :, :], ALU.mult)
                                if l > 0:
                                    nc.vector.tensor_add(msg[:, :cc * HID],
                                                         msg[:, :cc * HID],
                                                         wvt[:, :cc * HID])
                            mslice = lambda k: msg[:, k * HID:(k + 1) * HID]

                        for k in range(cc):
                            nc.tensor.matmul(po[:], lhsT=oh[:, k * 128:(k + 1) * 128],
                                             rhs=mslice(k),
                                             start=(done + k == 0),
                                             stop=(done + k == n_ch - 1))
                        done += cc

                    outsb = sb_out.tile([128, HID], F32, name="outsb")
                    nc.vector.tensor_scalar(out=outsb[:], in0=po[:],
                                            scalar1=dinvL_s[:, t:t + 1], scalar2=None,
                                            op0=ALU.mult)
                    if ell < 3:
                        outbf = sb_out.tile([128, HID], BF16, name="outbf")
                        nc.vector.tensor_copy(outbf[:], outsb[:])
                        nc.sync.dma_start(agin[agidx][t * 128:(t + 1) * 128, :], outbf[:])
                        build_q_from_out(outsb, t, ell - 1)
                        if debug:
                            dtgt = dbg_out1 if ell == 1 else dbg_out2
                            nc.sync.dma_start(dtgt[t * 128:(t + 1) * 128, :], outsb[:])
                    else:
                        final_tile(outsb, t)

            def final_tile(outsb, t):
                htp = ps_big.tile([HID, 512], F32, name="htp", space="PSUM")
                nc.tensor.transpose(htp[:, :128], in_=outsb[:], identity=iden[:])
                hts = sb_ht.tile([HID, 512], F32, name="hts")
                nc.vector.tensor_copy(hts[:, :128], htp[:, :128])
                yp = ps_kv.tile([128, 4 * HID], F32, name="kp", space="PSUM")
                nc.tensor.matmul(yp[:, :OUTC], lhsT=hts[:, :128], rhs=wout_s[:],
                                 start=True, stop=False)
                nc.tensor.matmul(yp[:, :OUTC], lhsT=ones_r[:], rhs=bout_s[:],
                                 start=False, stop=True)
                ysb = sb_out.tile([128, OUTC], F32, name="ysb")
                nc.vector.tensor_copy(ysb[:], yp[:, :OUTC])
                nc.sync.dma_start(y[t * 128:(t + 1) * 128, :], ysb[:])

            # ================= schedule =================
            build_slice(0)
            edge_layer(1, 0)
            nc.gpsimd.collective_compute(
                "AllGather", mybir.AluOpType.bypass,
                replica_groups=[list(range(ncore))],
                ins=[agin[0].opt()], outs=[agout[0].opt()])
            build_slice(1)
            edge_layer(2, 1)
            nc.gpsimd.collective_compute(
                "AllGather", mybir.AluOpType.bypass,
                replica_groups=[list(range(ncore))],
                ins=[agin[1].opt()], outs=[agout[1].opt()])
            build_slice(2)
            edge_layer(3, None)

    nc.compile()
    return nc


def assemble_output(cfg, results):
    n = cfg["n_nodes"]
    full = np.concatenate([results[c]["y"] for c in range(cfg["n_cores"])], axis=0)
    return full[:n]


# ======================= harness entry point =======================
LAST_EXEC_NS = [None]
LAST_RESULT = [None]


def kernel(**inputs):
    """Full (unsharded) inputs -> full [N, 16] float32 output.

    Shards edges by destination range across the 8 NeuronCores, compiles
    the SPMD Bass kernel for this edge layout, runs it, and reassembles
    the per-core output shards.
    """
    import os
    from concourse.bass_utils import run_bass_kernel_spmd

    x = np.asarray(inputs["x"], np.float32)
    edge_index = np.asarray(inputs["edge_index"])
    cfg = make_cfg(x.shape[0], edge_index, n_cores=8)
    in_maps = prep_inputs(
        cfg, x,
        inputs["W_lin"], inputs["b_lin"],
        inputs["Wq"], inputs["bq"],
        inputs["Wk"], inputs["bk"],
        inputs["Wv"], inputs["bv"],
        inputs["W_out"], inputs["b_out"],
    )
    nc = build_kernel(cfg)
    res = run_bass_kernel_spmd(nc, in_maps, core_ids=list(range(cfg["n_cores"])))
    LAST_EXEC_NS[0] = res.exec_time_ns
    LAST_RESULT[0] = res
    return assemble_output(cfg, res.results)



# revision 2
# speedup vs baseline: 1.0181x; 1.0181x over previous
"""DNANet Bass kernel v3.

Changes vs v2:
- Node->slot remap: nodes degree-sorted and snake-dealt across cores so
  per-core edge counts (and per-tile chunk counts) match across cores;
  kvtab rows are in slot order.
- Each core builds k/v only for its OWN nodes (interleaved into the edge
  pass, sharing the q-build transpose), then the k/v table slice is
  AllGathered (in two halves, first half overlapped with the edge pass)
  and copied into the interleaved kvtab columns.  This removes the
  ~0.9ms all-nodes table rebuild between layers.
- bk dropped entirely (softmax over layers is invariant to the q.bk
  shift); bv folded into a host-precomputed rank-1 per-node correction
  (sum of attention weights is 1), applied with one fused
  scalar_tensor_tensor per tile.
- q tiles stay resident in SBUF (no DRAM round trip).
- Gather pad indices are negative (skipped by SWDGE) to trim Q7
  descriptor-generation time.
- onehot inputs in bf16 for 2x DVE rate; PSUM->SBUF casts moved to the
  (otherwise idle) scalar engine.
"""
import math
import numpy as np
import concourse.bacc as bacc
import concourse.mybir as mybir
import concourse.tile as tile
from concourse.masks import make_identity

F32, I32, I16 = mybir.dt.float32, mybir.dt.int32, mybir.dt.int16
BF16 = mybir.dt.bfloat16
HID, H, DH, INC, OUTC = 64, 4, 16, 128, 16
LMAX = 3
KVROW = LMAX * 2 * HID  # 384 elements per node row in kv table
HALF = 25088            # row split so int16 indices stay positive
C_MAX = 8               # chunks per compute superchunk
GW = 8                  # chunks per gather window (<=1024 idx per dma_gather)


def _wrap16(arr_cm):
    """[tot_ch, 128] int -> [128, tot_ch*8] int16 ucode idx layout."""
    tc_, _ = arr_cm.shape
    a = arr_cm.reshape(tc_, 8, 16).transpose(2, 0, 1).reshape(16, tc_ * 8)
    out = np.zeros((128, tc_ * 8), np.int16)
    for r in range(8):
        out[r * 16:(r + 1) * 16] = a
    return out


def make_cfg(n_nodes, edge_index, n_cores=8):
    tpc = math.ceil(n_nodes / n_cores / 128)
    npad = n_cores * tpc * 128
    nsh = tpc * 128
    half = HALF if npad + 128 > 32000 else npad + 128

    src_e = np.asarray(edge_index[0], dtype=np.int64)
    dst_e = np.asarray(edge_index[1], dtype=np.int64)
    loop = np.arange(n_nodes, dtype=np.int64)
    src = np.concatenate([src_e, loop])
    dst = np.concatenate([dst_e, loop])

    deg = np.bincount(dst, minlength=n_nodes).astype(np.float32)
    dinv = np.where(deg > 0, 1.0 / np.sqrt(np.maximum(deg, 1.0)), 0.0).astype(np.float32)

    # ---- node -> (core, slot) snake deal by degree --------------------
    order = np.argsort(-deg, kind="stable")  # high degree first
    slot_of = np.full(npad, -1, np.int64)
    core_nodes = [[] for _ in range(n_cores)]
    for r in range(0, n_nodes, n_cores):
        blk = order[r:r + n_cores]
        cs = range(n_cores) if (r // n_cores) % 2 == 0 else range(n_cores - 1, -1, -1)
        for c, nd in zip(cs, blk):
            core_nodes[c].append(nd)
    for c in range(n_cores):
        cn = np.asarray(core_nodes[c], np.int64)
        slot_of[cn] = c * nsh + np.arange(len(cn))
        core_nodes[c] = cn
    # dinv in slot order (pad slots -> 0)
    dinv_slot = np.zeros(npad, np.float32)
    for c in range(n_cores):
        dinv_slot[c * nsh:c * nsh + len(core_nodes[c])] = dinv[core_nodes[c]]

    # chunk path uses only the original edges; the added self-loops are
    # handled by the batched self path (own k/v rows, no gather)
    src_sl = slot_of[src_e]
    dst_sl = slot_of[dst_e]

    # bias correction: corr[n] = dinv[n] * sum_{e->n} dinv[src]
    ssum = np.zeros(n_nodes, np.float64)
    np.add.at(ssum, dst, dinv[src].astype(np.float64))
    corr = dinv * ssum.astype(np.float32)
    corr_slot = np.zeros(npad, np.float32)
    for c in range(n_cores):
        corr_slot[c * nsh:c * nsh + len(core_nodes[c])] = corr[core_nodes[c]]

    key = dst_sl * 2 + (src_sl >= half)
    order_e = np.argsort(key, kind="stable")
    src_s, dst_s = src_sl[order_e], dst_sl[order_e]

    cnt_lo = np.zeros((n_cores, tpc), np.int64)
    cnt_hi = np.zeros((n_cores, tpc), np.int64)
    core_of = dst_s // nsh
    ltile = (dst_s % nsh) // 128
    is_hi = src_s >= half
    np.add.at(cnt_lo, (core_of[~is_hi], ltile[~is_hi]), 1)
    np.add.at(cnt_hi, (core_of[is_hi], ltile[is_hi]), 1)
    kt_lo = np.maximum(np.ceil(cnt_lo.max(axis=0) / 128).astype(np.int64), 1)
    kt_hi = np.ceil(cnt_hi.max(axis=0) / 128).astype(np.int64)
    kt = kt_lo + kt_hi
    tot_ch = int(kt.sum())
    chunk_base = np.concatenate([[0], np.cumsum(kt)]).astype(np.int64)

    kvidx_cm = np.zeros((n_cores, tot_ch, 128), np.int64)  # pad -> row 0 (masked)
    dloc_cm = np.full((n_cores, tot_ch, 128), 128, np.int64)
    for c in range(n_cores):
        lo = np.searchsorted(dst_s, c * nsh)
        hi_ = np.searchsorted(dst_s, (c + 1) * nsh)
        sseg_c = src_s[lo:hi_]
        dseg_c = dst_s[lo:hi_]
        for t in range(tpc):
            g0 = c * nsh + t * 128
            l0 = np.searchsorted(dseg_c, g0)
            l1 = np.searchsorted(dseg_c, g0 + 128)
            if l1 == l0:
                continue
            sseg = sseg_c[l0:l1]
            dseg = dseg_c[l0:l1]
            hseg = sseg >= half
            for half_i, mask, base_ch in (
                (0, ~hseg, chunk_base[t]),
                (1, hseg, chunk_base[t] + kt_lo[t]),
            ):
                sv = sseg[mask]
                dv = dseg[mask]
                n = len(sv)
                if n == 0:
                    continue
                ch = base_ch + np.arange(n) // 128
                lane = np.arange(n) % 128
                kvidx_cm[c, ch, lane] = sv - (half if half_i else 0)
                dloc_cm[c, ch, lane] = dv - g0

    # gather windows (same enumeration as edge_layer) + per-core valid counts
    scs_all = []  # (tile, chunk_base+offset, cc) in issue order
    for t in range(tpc):
        for seg_o, seg_n in ((0, kt_lo[t]), (kt_lo[t], kt_hi[t])):
            o = 0
            while o < seg_n:
                w = min(GW, seg_n - o)
                scs_all.append((t, int(chunk_base[t]) + seg_o + o, w))
                o += w
    n_g = len(scs_all)

    return dict(
        n_cores=n_cores, tpc=tpc, npad=npad, nsh=nsh, n_nodes=n_nodes,
        half=half,
        kt=[int(k) for k in kt], kt_lo=[int(k) for k in kt_lo],
        kt_hi=[int(k) for k in kt_hi],
        tot_ch=tot_ch, chunk_base=[int(b) for b in chunk_base],
        n_g=n_g,
        kvidx=np.stack([_wrap16(kvidx_cm[c].astype(np.int16)) for c in range(n_cores)]),
        dlocT=np.ascontiguousarray(
            dloc_cm.astype(np.int32).transpose(0, 2, 1)),  # [c, 128, tot_ch]
        dinv_slot=dinv_slot, corr_slot=corr_slot,
        core_nodes=core_nodes,
    )


def prep_inputs(cfg, x, W_lin, b_lin, Wq, bq, Wk, bk, Wv, bv, W_out, b_out):
    ncore, npad, tpc, nsh = cfg["n_cores"], cfg["npad"], cfg["tpc"], cfg["nsh"]
    x = np.asarray(x, np.float32)
    bv = np.asarray(bv, np.float32)
    import ml_dtypes
    iota = np.broadcast_to(np.arange(128, dtype=np.float32), (128, 128))
    base = {
        "wlin": np.asarray(W_lin, np.float32),
        "wq": np.asarray(Wq, np.float32),
        "wk": np.asarray(Wk, np.float32),
        "wv": np.asarray(Wv, np.float32),
        "wout": np.asarray(W_out, np.float32),
        "blin_col": np.asarray(b_lin, np.float32).reshape(HID, 1),
        "bq_r": np.asarray(bq, np.float32).reshape(1, HID),
        "bout_r": np.asarray(b_out, np.float32).reshape(1, OUTC),
        "iota_bf": iota.astype(ml_dtypes.bfloat16),  # values 0..127 exact in bf16
    }
    in_maps = []
    for c in range(ncore):
        m = dict(base)
        cn = cfg["core_nodes"][c]
        xTL = np.zeros((INC, nsh), np.float32)
        xTL[:, :len(cn)] = x[cn].T
        m["xTL"] = xTL
        dsl = cfg["dinv_slot"][c * nsh:(c + 1) * nsh]
        m["dinvL"] = np.ascontiguousarray(dsl.reshape(tpc, 128).T)  # [128, tpc]
        csl = cfg["corr_slot"][c * nsh:(c + 1) * nsh]
        # biasT[p, t*64+j] = corr[slot t*128+p] * bv[j]
        bt = csl.reshape(tpc, 128)[:, :, None] * bv[None, None, :]  # [tpc,128,64]
        m["biasT"] = np.ascontiguousarray(
            bt.transpose(1, 0, 2).reshape(128, tpc * HID)).astype(np.float32)
        m["kvidx"] = cfg["kvidx"][c]
        # dloc as bf16 (values 0..128 exact)
        m["dlocT"] = cfg["dlocT"][c].astype(ml_dtypes.bfloat16)
        in_maps.append(m)
    return in_maps


def build_kernel(cfg):
    ncore, tpc, npad, nsh = cfg["n_cores"], cfg["tpc"], cfg["npad"], cfg["nsh"]
    kt, kt_lo, kt_hi = cfg["kt"], cfg["kt_lo"], cfg["kt_hi"]
    tot_ch, chunk_base, half = cfg["tot_ch"], cfg["chunk_base"], cfg["half"]

    # AllGather quarters: tile group boundaries
    qb = [0, tpc // 4, tpc // 2, 3 * tpc // 4, tpc]
    rows_h = [(qb[i + 1] - qb[i]) * 128 for i in range(4)]
    NQ = 4

    nc = bacc.Bacc("TRN2", target_bir_lowering=False, debug=False,
                   num_devices=ncore, num_swdge_queues=2)

    xTL = nc.dram_tensor("xTL", [INC, nsh], F32, kind="ExternalInput")
    wlin = nc.dram_tensor("wlin", [INC, HID], F32, kind="ExternalInput")
    wq = nc.dram_tensor("wq", [HID, HID], F32, kind="ExternalInput")
    wk = nc.dram_tensor("wk", [HID, HID], F32, kind="ExternalInput")
    wv = nc.dram_tensor("wv", [HID, HID], F32, kind="ExternalInput")
    wout = nc.dram_tensor("wout", [HID, OUTC], F32, kind="ExternalInput")
    blin_col = nc.dram_tensor("blin_col", [HID, 1], F32, kind="ExternalInput")
    bq_r = nc.dram_tensor("bq_r", [1, HID], F32, kind="ExternalInput")
    bout_r = nc.dram_tensor("bout_r", [1, OUTC], F32, kind="ExternalInput")
    iota_bf = nc.dram_tensor("iota_bf", [128, 128], BF16, kind="ExternalInput")
    dinvL_d = nc.dram_tensor("dinvL", [128, tpc], F32, kind="ExternalInput")
    biasT_d = nc.dram_tensor("biasT", [128, tpc * HID], F32, kind="ExternalInput")
    kvidx_d = nc.dram_tensor("kvidx", [128, tot_ch * 8], I16, kind="ExternalInput")
    dlocT_d = nc.dram_tensor("dlocT", [128, tot_ch], BF16, kind="ExternalInput")
    y = nc.dram_tensor("y", [nsh, OUTC], F32, kind="ExternalOutput")

    with tile.TileContext(nc) as tc:
        import contextlib
        ctx = contextlib.ExitStack()
        with ctx:
            cpool = ctx.enter_context(tc.tile_pool(name="const", bufs=1))
            dram = ctx.enter_context(tc.tile_pool(name="dram", bufs=1, space="DRAM"))

            kvtab = dram.tile([npad + 128, KVROW], BF16, name="kvtab")
            kvown = dram.tile([nsh, KVROW], BF16, name="kvown")
            # per (slice, quarter) collective buffers
            agin = [[dram.tile([rows_h[h], 128], BF16, name=f"agin{s}_{h}")
                     for h in range(NQ)] for s in (0, 1, 2)]
            agout = [[dram.tile([ncore * rows_h[h], 128], BF16,
                                name=f"agout{s}_{h}", addr_space="Shared")
                      for h in range(NQ)] for s in (0, 1, 2)]

            def load_const(dt_, shape, src_ap, name):
                t_ = cpool.tile(shape, dt_, name=name)
                nc.sync.dma_start(t_[:], src_ap)
                return t_

            wlin_s = load_const(F32, [INC, HID], wlin[:], "wlin_s")
            wq_s = load_const(F32, [HID, HID], wq[:], "wq_s")
            wk_s = load_const(F32, [HID, HID], wk[:], "wk_s")
            wv_s = load_const(F32, [HID, HID], wv[:], "wv_s")
            wout_s = load_const(F32, [HID, OUTC], wout[:], "wout_s")
            blin_s = load_const(F32, [HID, 1], blin_col[:], "blin_s")
            bq_s = load_const(F32, [1, HID], bq_r[:], "bq_s")
            bout_s = load_const(F32, [1, OUTC], bout_r[:], "bout_s")
            iota_s = load_const(BF16, [128, 128], iota_bf[:], "iota_s")
            dinvL_s = load_const(F32, [128, tpc], dinvL_d[:], "dinvL_s")
            biasT_s = load_const(F32, [128, tpc * HID], biasT_d[:], "biasT_s")
            iden = cpool.tile([128, 128], F32, name="iden")
            make_identity(nc, iden[:])
            iden_bf = cpool.tile([128, 128], BF16, name="iden_bf")
            nc.vector.tensor_copy(iden_bf[:], iden[:])
            ones_r = cpool.tile([1, 128], F32, name="ones_r")
            nc.vector.memset(ones_r[:], 1.0)
            # SBUF-resident q tiles for layers 2 and 3
            qstore = [cpool.tile([128, tpc * HID], BF16, name=f"qstore{i}")
                      for i in (0, 1)]

            sb_xt = ctx.enter_context(tc.tile_pool(name="sb_xt", bufs=3))
            sb_ht = ctx.enter_context(tc.tile_pool(name="sb_ht", bufs=3))
            sb_kv = ctx.enter_context(tc.tile_pool(name="sb_kv", bufs=3))
            sb_idx = ctx.enter_context(tc.tile_pool(name="sb_idx", bufs=6))
            sb_oh = ctx.enter_context(tc.tile_pool(name="sb_oh", bufs=4))
            sb_g = ctx.enter_context(tc.tile_pool(name="sb_g", bufs=4))
            sb_ve = ctx.enter_context(tc.tile_pool(name="sb_ve", bufs=4))
            sb_sm = ctx.enter_context(tc.tile_pool(name="sb_sm", bufs=4))
            sb_out = ctx.enter_context(tc.tile_pool(name="sb_out", bufs=3))
            sb_self = ctx.enter_context(tc.tile_pool(name="sb_self", bufs=2))
            sb_self2 = ctx.enter_context(tc.tile_pool(name="sb_self2", bufs=2))
            ps_big = ctx.enter_context(tc.tile_pool(name="ps_big", bufs=1, space="PSUM"))
            ps_kv = ctx.enter_context(tc.tile_pool(name="ps_kv", bufs=2, space="PSUM"))
            ps_out = ctx.enter_context(tc.tile_pool(name="ps_out", bufs=2, space="PSUM"))
            ps_qe = ctx.enter_context(tc.tile_pool(name="ps_qe", bufs=2, space="PSUM"))
            ps_oht = ctx.enter_context(tc.tile_pool(name="ps_oht", bufs=1, space="PSUM"))

            AF, ALU = mybir.ActivationFunctionType, mybir.AluOpType
            AX = mybir.AxisListType
            qctr = [0]

            def next_q():
                qctr[0] += 1
                return qctr[0] % 2

            def agslice(s, t):
                """(quarter, row0) for tile t in the slice-s AG input."""
                for h in range(NQ):
                    if t < qb[h + 1]:
                        return h, (t - qb[h]) * 128
                raise AssertionError(t)

            # ---- per-own-tile k/v (+q) build from hts [HID, 128] -------
            def own_kv_build(s, t, hts, hcols=0):
                kvp = ps_kv.tile([128, 2 * HID], F32, name="kvp", space="PSUM")
                nc.tensor.matmul(kvp[:, :HID], lhsT=hts[:, hcols:hcols + 128],
                                 rhs=wk_s[:], start=True, stop=True)
                nc.tensor.matmul(kvp[:, HID:], lhsT=hts[:, hcols:hcols + 128],
                                 rhs=wv_s[:], start=True, stop=True)
                kvsb = sb_kv.tile([128, 128], BF16, name="kvsb")
                nc.scalar.copy(kvsb[:, :HID], kvp[:, :HID])
                nc.scalar.mul(kvsb[:, HID:], kvp[:, HID:],
                              dinvL_s[:, t:t + 1])
                h_, r0 = agslice(s, t)
                nc.sync.dma_start(agin[s][h_][r0:r0 + 128, :], kvsb[:])
                nc.sync.dma_start(
                    kvown[t * 128:(t + 1) * 128, s * 128:(s + 1) * 128], kvsb[:])

            def own_q_build(qi, t, hts):
                qp = ps_kv.tile([128, 2 * HID], F32, name="kvp", space="PSUM")
                nc.tensor.matmul(qp[:, :HID], lhsT=hts[:, :128], rhs=wq_s[:],
                                 start=True, stop=False)
                nc.tensor.matmul(qp[:, :HID], lhsT=ones_r[:], rhs=bq_s[:],
                                 start=False, stop=True)
                nc.scalar.copy(qstore[qi][:, t * HID:(t + 1) * HID], qp[:, :HID])

            def fire_ag(s, h_):
                nc.gpsimd.collective_compute(
                    "AllGather", mybir.AluOpType.bypass,
                    replica_groups=[list(range(ncore))],
                    ins=[agin[s][h_].opt()], outs=[agout[s][h_].opt()])

            def copy_ag(s, h_):
                # copy into kvtab columns [s*128:(s+1)*128], rows by core block
                rh = rows_h[h_]
                off = qb[h_] * 128
                src = agout[s][h_][:].rearrange("(c r) d -> c r d", c=ncore)
                dst = kvtab[0:npad, s * 128:(s + 1) * 128].rearrange(
                    "(c r) d -> c r d", c=ncore, r=nsh)[:, off:off + rh, :]
                nc.sync.dma_start(dst, src)

            # ================= slice 0 (h = x @ W_lin + b) ==============
            def build_slice0():
                ng = nsh // 512  # 12 full groups of 512
                rem = (nsh - ng * 512) // 128
                for g in range(ng + 1):
                    nb = 4 if g < ng else rem
                    if nb == 0:
                        continue
                    w = nb * 128
                    xt_t = sb_xt.tile([INC, 512], F32, name="xt_t")
                    nc.sync.dma_start(xt_t[:, :w], xTL[:, g * 512:g * 512 + w])
                    htp = ps_big.tile([HID, 512], F32, name="htp", space="PSUM")
                    for b in range(nb):
                        nc.tensor.matmul(htp[:, b * 128:(b + 1) * 128], lhsT=wlin_s[:],
                                         rhs=xt_t[:, b * 128:(b + 1) * 128],
                                         start=True, stop=True)
                    hts = sb_ht.tile([HID, 512], F32, name="hts")
                    nc.vector.tensor_scalar(out=hts[:, :w], in0=htp[:, :w],
                                            scalar1=blin_s[:], scalar2=None,
                                            op0=ALU.add)
                    for b in range(nb):
                        t_ = g * 4 + b
                        own_kv_build(0, t_, hts, hcols=b * 128)
                        if t_ + 1 in qb[1:4]:
                            h_ = qb.index(t_ + 1) - 1
                            fire_ag(0, h_)
                            copy_ag(0, h_)

            HT = (tpc + 1) // 2  # tiles per self-batch half
            SELF_T = [(0, HT), (HT, tpc - HT)]

            def self_msgs(ell, hf):
                """Batched self-loop messages for tiles [t0, t0+nb).

                Returns (tile, stride, off): per-tile message is
                tile[:, (t-t0)*stride + off : ... + HID] (bf16, already
                premultiplied by dinv[own])."""
                t0, nb = SELF_T[hf]
                if ell == 1:
                    sv1 = sb_self2.tile([128, HT * HID], BF16, name="sv1")
                    nc.sync.dma_start(
                        sv1[:, :nb * HID].rearrange("p (b d) -> p b d", b=nb),
                        kvown[:].rearrange("(b p) e -> p b e", p=128)
                        [:, t0:t0 + nb, HID:2 * HID])
                    return sv1, HID, 0
                skv = sb_self.tile([128, HT * LMAX * 128], BF16, name="skv")
                nc.sync.dma_start(
                    skv[:, :nb * ell * 128].rearrange("p (b e) -> p b e", b=nb),
                    kvown[:].rearrange("(b p) e -> p b e", p=128)
                    [:, t0:t0 + nb, :ell * 128])
                kvvS = skv[:, :nb * ell * 128].rearrange(
                    "p (b l s h d) -> p b l s h d", b=nb, l=ell, s=2, h=H, d=DH)
                qbv = qstore[ell % 2][:, t0 * HID:(t0 + nb) * HID].rearrange(
                    "p (b u h d) -> p b u h d", b=nb, u=1, h=H).to_broadcast(
                    [128, nb, ell, H, DH])
                qks = sb_self2.tile([128, HT * LMAX * HID], BF16, name="qks")
                qksv = qks[:, :nb * ell * HID].rearrange(
                    "p (b l h d) -> p b l h d", b=nb, l=ell, h=H, d=DH)
                nc.vector.tensor_tensor(qksv, qbv, kvvS[:, :, :, 0, :, :], ALU.mult)
                ssS = sb_sm.tile([128, HT * LMAX * H], F32, name="ssS")
                nc.vector.reduce_sum(
                    ssS[:, :nb * ell * H].rearrange("p (b l h) -> p b l h",
                                                    b=nb, l=ell, h=H),
                    qksv, axis=AX.X)
                eS = sb_sm.tile([128, HT * LMAX * H], BF16, name="eS")
                nc.scalar.activation(eS[:, :nb * ell * H], ssS[:, :nb * ell * H],
                                     AF.Exp, scale=1.0 / math.sqrt(DH))
                denS = sb_sm.tile([128, HT * H], F32, name="denS")
                nc.vector.reduce_sum(
                    denS[:, :nb * H].rearrange("p (b h) -> p b h", b=nb),
                    eS[:, :nb * ell * H].rearrange("p (b l h) -> p b h l",
                                                   b=nb, l=ell, h=H),
                    axis=AX.X)
                rdenS = sb_sm.tile([128, HT * H], BF16, name="rdenS")
                with nc.allow_low_precision(reason="bf16 softmax weights"):
                    nc.vector.reciprocal(rdenS[:, :nb * H], denS[:, :nb * H])
                attS = sb_sm.tile([128, HT * LMAX * H], BF16, name="attS")
                atv = attS[:, :nb * ell * H].rearrange("p (b l h) -> p b l h",
                                                       b=nb, l=ell, h=H)
                nc.vector.tensor_tensor(
                    atv,
                    eS[:, :nb * ell * H].rearrange("p (b l h) -> p b l h",
                                                   b=nb, l=ell, h=H),
                    rdenS[:, :nb * H].rearrange("p (b u h) -> p b u h", b=nb, u=1)
                    .to_broadcast([128, nb, ell, H]),
                    ALU.mult)
                msgS = sb_self2.tile([128, HT * HID], BF16, name="msgS")
                wvS = sb_self2.tile([128, HT * HID], BF16, name="wvS")
                mS = msgS[:, :nb * HID].rearrange("p (b h d) -> p b h d", b=nb, h=H)
                wS = wvS[:, :nb * HID].rearrange("p (b h d) -> p b h d", b=nb, h=H)
                for l in range(ell):
                    nc.vector.tensor_tensor(
                        mS if l == 0 else wS,
                        atv[:, :, l, :].to_broadcast([128, nb, H, DH]),
                        kvvS[:, :, l, 1, :, :], ALU.mult)
                    if l > 0:
                        nc.vector.tensor_add(msgS[:, :nb * HID],
                                             msgS[:, :nb * HID],
                                             wvS[:, :nb * HID])
                return msgS, HID, 0

            def attn_block(ell, t, kvg, oo, cc, oh):
                L = ell
                qtile = qstore[ell % 2][:, t * HID:(t + 1) * HID]
                qe = ps_qe.tile([128, C_MAX * HID], F32, name="qe",
                                space="PSUM")
                for k in range(cc):
                    ohtp = ps_oht.tile([128, 128], BF16, name="ohtp",
                                       space="PSUM")
                    nc.tensor.transpose(ohtp[:],
                                        in_=oh[:, k * 128:(k + 1) * 128],
                                        identity=iden_bf[:])
                    ohts = sb_oh.tile([128, 128], BF16, name="ohts")
                    nc.scalar.copy(ohts[:], ohtp[:])
                    nc.tensor.matmul(qe[:, k * HID:(k + 1) * HID],
                                     lhsT=ohts[:], rhs=qtile,
                                     start=True, stop=True)

                qeb = sb_ve.tile([128, C_MAX * HID], BF16, name="qeb")
                nc.scalar.copy(qeb[:, :cc * HID], qe[:, :cc * HID])
                kvv = kvg[:, oo * L * 128:(oo + cc) * L * 128].rearrange(
                    "p (c l s h d) -> p c l s h d", c=cc, l=L, s=2, h=H, d=DH)
                qv = qeb[:, :cc * HID].rearrange("p (c h d) -> p c h d",
                                                 h=H, d=DH)
                satt = sb_sm.tile([128, C_MAX * H * LMAX], F32, name="satt")
                sv = satt[:, :cc * H * L].rearrange("p (c h l) -> p c h l",
                                                    h=H, l=L)
                qk = sb_ve.tile([128, C_MAX * HID], BF16, name="qk")
                qkv = qk[:, :cc * HID].rearrange("p (c h d) -> p c h d",
                                                 h=H, d=DH)
                for l in range(L):
                    nc.vector.tensor_tensor(qkv, qv, kvv[:, :, l, 0, :, :],
                                            ALU.mult)
                    nc.vector.reduce_sum(sv[:, :, :, l], qkv, axis=AX.X)
                eatt = sb_sm.tile([128, C_MAX * H * LMAX], BF16, name="eatt")
                nc.scalar.activation(eatt[:, :cc * H * L], satt[:, :cc * H * L],
                                     AF.Exp, scale=1.0 / math.sqrt(DH))
                den = sb_sm.tile([128, C_MAX * H], F32, name="den")
                nc.vector.reduce_sum(
                    den[:, :cc * H].rearrange("p (c h) -> p c h", h=H),
                    eatt[:, :cc * H * L].rearrange("p (c h l) -> p c h l",
                                                   h=H, l=L),
                    axis=AX.X)
                rden = sb_sm.tile([128, C_MAX * H], BF16, name="rden")
                with nc.allow_low_precision(
                        reason="bf16 softmax weights; rel-err gate 2e-2"):
                    nc.vector.reciprocal(rden[:, :cc * H], den[:, :cc * H])
                att = sb_sm.tile([128, C_MAX * H * LMAX], BF16, name="att")
                av = att[:, :cc * H * L].rearrange("p (c h l) -> p c h l",
                                                   h=H, l=L)
                nc.vector.tensor_tensor(
                    av,
                    eatt[:, :cc * H * L].rearrange("p (c h l) -> p c h l",
                                                   h=H, l=L),
                    rden[:, :cc * H].rearrange("p (c h u) -> p c h u",
                                               h=H, u=1)
                    .to_broadcast([128, cc, H, L]),
                    ALU.mult)
                msg = sb_ve.tile([128, C_MAX * HID], BF16, name="msg")
                wvt = sb_ve.tile([128, C_MAX * HID], BF16, name="wvt")
                mv = msg[:, :cc * HID].rearrange("p (c h d) -> p c h d",
                                                 h=H, d=DH)
                wvv = wvt[:, :cc * HID].rearrange("p (c h d) -> p c h d",
                                                  h=H, d=DH)
                for l in range(L):
                    nc.vector.tensor_tensor(
                        mv if l == 0 else wvv,
                        av[:, :, :, l].to_broadcast([128, cc, H, DH]),
                        kvv[:, :, l, 1, :, :], ALU.mult)
                    if l > 0:
                        nc.vector.tensor_add(msg[:, :cc * HID],
                                             msg[:, :cc * HID],
                                             wvt[:, :cc * HID])
                return lambda k: msg[:, k * HID:(k + 1) * HID]

            # ================= edge pass =================
            def edge_layer(ell):
                L = ell
                selfb = [self_msgs(ell, 0), self_msgs(ell, 1)]
                for t in range(tpc):
                    stile, sstride, soff = selfb[0 if t < HT else 1]
                    st0 = 0 if t < HT else HT
                    po = ps_out.tile([128, HID], F32, name="po", space="PSUM")
                    n_ch = kt[t]
                    base = chunk_base[t]
                    done = 0
                    wins = []
                    for seg_o, seg_n in ((0, kt_lo[t]), (kt_lo[t], kt_hi[t])):
                        o = 0
                        while o < seg_n:
                            w = min(GW, seg_n - o)
                            wins.append((seg_o + o, w))
                            o += w
                    for (wo, gw) in wins:
                        rb = 0 if wo < kt_lo[t] else half
                        wcb = base + wo
                        kvi = sb_idx.tile([128, GW * 8], I16, name="kvi")
                        nc.sync.dma_start(kvi[:, :gw * 8],
                                          kvidx_d[:, wcb * 8:(wcb + gw) * 8])
                        if ell == 1:
                            gt = sb_g.tile([128, GW * LMAX * 128], BF16, name="kvg")
                            nc.gpsimd.dma_gather(
                                out_ap=gt[:, :gw * 128].rearrange(
                                    "p (n d) -> p n d", d=128),
                                in_ap=kvtab[rb:, 0:128],
                                idxs_ap=kvi[:, :gw * 8],
                                num_idxs=gw * 128, num_idxs_reg=gw * 128,
                                elem_size=128, elem_step=KVROW,
                                queue_num=next_q())
                        else:
                            gt = sb_g.tile([128, GW * LMAX * 128], BF16, name="kvg")
                            nc.gpsimd.dma_gather(
                                out_ap=gt[:, :gw * L * 128].rearrange(
                                    "p (n d) -> p n d", d=L * 128),
                                in_ap=kvtab[rb:, :L * 128],
                                idxs_ap=kvi[:, :gw * 8],
                                num_idxs=gw * 128, num_idxs_reg=gw * 128,
                                elem_size=L * 128, elem_step=KVROW,
                                queue_num=next_q())
                        oo = 0
                        while oo < gw:
                            cc = min(C_MAX, gw - oo)
                            cb = wcb + oo
                            dli = sb_idx.tile([128, C_MAX], BF16, name="dli")
                            nc.sync.dma_start(dli[:, :cc], dlocT_d[:, cb:cb + cc])
                            oh = sb_oh.tile([128, C_MAX * 128], BF16, name="oh")
                            nc.vector.tensor_tensor(
                                oh[:, :cc * 128].rearrange("p (c i) -> p c i",
                                                           c=cc, i=128),
                                dli[:, :cc].rearrange("p (c u) -> p c u", c=cc, u=1)
                                .to_broadcast([128, cc, 128]),
                                iota_s[:].rearrange("p (u i) -> p u i", u=1, i=128)
                                .to_broadcast([128, cc, 128]),
                                ALU.is_equal)
                            if ell == 1:
                                vgv = gt[:, oo * 128:(oo + cc) * 128].rearrange(
                                    "p (c s d) -> p c s d", s=2, d=HID)
                                mslice = lambda k, vgv=vgv: vgv[:, k, 1, :]
                            else:
                                mslice = attn_block(ell, t, gt, oo, cc, oh)
                            for k in range(cc):
                                nc.tensor.matmul(po[:],
                                                 lhsT=oh[:, k * 128:(k + 1) * 128],
                                                 rhs=mslice(k),
                                                 start=(done + k == 0),
                                                 stop=(done + k == n_ch - 1))
                            done += cc
                            oo += cc

                    # out = (po + self) * dinv[dst] + corr[dst] * bv  (2 fused ops)
                    so = (t - st0) * sstride + soff
                    ob0 = sb_out.tile([128, HID], F32, name="ob0")
                    nc.vector.scalar_tensor_tensor(
                        out=ob0[:], in0=stile[:, so:so + HID],
                        scalar=dinvL_s[:, t:t + 1],
                        in1=biasT_s[:, t * HID:(t + 1) * HID],
                        op0=ALU.mult, op1=ALU.add)
                    outsb = sb_out.tile([128, HID], F32, name="outsb")
                    nc.vector.scalar_tensor_tensor(
                        out=outsb[:], in0=po[:], scalar=dinvL_s[:, t:t + 1],
                        in1=ob0[:], op0=ALU.mult, op1=ALU.add)
                    if ell < 3:
                        # shared transpose for q-build and kv-build
                        htp = ps_big.tile([HID, 512], F32, name="htp", space="PSUM")
                        nc.tensor.transpose(htp[:, :128], in_=outsb[:], identity=iden[:])
                        hts = sb_ht.tile([HID, 512], F32, name="hts")
                        nc.scalar.copy(hts[:, :128], htp[:, :128])
                        own_q_build((ell + 1) % 2, t, hts)
                        own_kv_build(ell, t, hts)
                        if t + 1 in qb[1:]:
                            h_ = qb.index(t + 1) - 1
                            fire_ag(ell, h_)
                            copy_ag(ell, h_)
                    else:
                        final_tile(outsb, t)

            def final_tile(outsb, t):
                htp = ps_big.tile([HID, 512], F32, name="htp", space="PSUM")
                nc.tensor.transpose(htp[:, :128], in_=outsb[:], identity=iden[:])
                hts = sb_ht.tile([HID, 512], F32, name="hts")
                nc.scalar.copy(hts[:, :128], htp[:, :128])
                yp = ps_kv.tile([128, 2 * HID], F32, name="kvp", space="PSUM")
                nc.tensor.matmul(yp[:, :OUTC], lhsT=hts[:, :128], rhs=wout_s[:],
                                 start=True, stop=False)
                nc.tensor.matmul(yp[:, :OUTC], lhsT=ones_r[:], rhs=bout_s[:],
                                 start=False, stop=True)
                ysb = sb_out.tile([128, OUTC], F32, name="ysb")
                nc.scalar.copy(ysb[:], yp[:, :OUTC])
                nc.sync.dma_start(y[t * 128:(t + 1) * 128, :], ysb[:])

            # ================= schedule =================
            build_slice0()
            fire_ag(0, 3)
            copy_ag(0, 3)
            edge_layer(1)
            edge_layer(2)
            edge_layer(3)

    nc.compile()
    return nc


def assemble_output(cfg, results):
    n = cfg["n_nodes"]
    out = np.zeros((n, OUTC), np.float32)
    for c in range(cfg["n_cores"]):
        cn = cfg["core_nodes"][c]
        out[cn] = results[c]["y"][:len(cn)]
    return out


# ======================= harness entry point =======================
LAST_EXEC_NS = [None]
LAST_RESULT = [None]


def kernel(**inputs):
    """Full (unsharded) inputs -> full [N, 16] float32 output."""
    from concourse.bass_utils import run_bass_kernel_spmd

    x = np.asarray(inputs["x"], np.float32)
    edge_index = np.asarray(inputs["edge_index"])
    cfg = make_cfg(x.shape[0], edge_index, n_cores=8)
    in_maps = prep_inputs(
        cfg, x,
        inputs["W_lin"], inputs["b_lin"],
        inputs["Wq"], inputs["bq"],
        inputs["Wk"], inputs["bk"],
        inputs["Wv"], inputs["bv"],
        inputs["W_out"], inputs["b_out"],
    )
    nc = build_kernel(cfg)
    res = run_bass_kernel_spmd(nc, in_maps, core_ids=list(range(cfg["n_cores"])))
    LAST_EXEC_NS[0] = res.exec_time_ns
    LAST_RESULT[0] = res
    return assemble_output(cfg, res.results)


# revision 3
# speedup vs baseline: 1.0220x; 1.0038x over previous
"""DNANet Bass kernel v3.

Changes vs v2:
- Node->slot remap: nodes degree-sorted and snake-dealt across cores so
  per-core edge counts (and per-tile chunk counts) match across cores;
  kvtab rows are in slot order.
- Each core builds k/v only for its OWN nodes (interleaved into the edge
  pass, sharing the q-build transpose), then the k/v table slice is
  AllGathered (in two halves, first half overlapped with the edge pass)
  and copied into the interleaved kvtab columns.  This removes the
  ~0.9ms all-nodes table rebuild between layers.
- bk dropped entirely (softmax over layers is invariant to the q.bk
  shift); bv folded into a host-precomputed rank-1 per-node correction
  (sum of attention weights is 1), applied with one fused
  scalar_tensor_tensor per tile.
- q tiles stay resident in SBUF (no DRAM round trip).
- Gather pad indices are negative (skipped by SWDGE) to trim Q7
  descriptor-generation time.
- onehot inputs in bf16 for 2x DVE rate; PSUM->SBUF casts moved to the
  (otherwise idle) scalar engine.
"""
import math
import numpy as np
import concourse.bacc as bacc
import concourse.mybir as mybir
import concourse.tile as tile
from concourse.masks import make_identity

F32, I32, I16 = mybir.dt.float32, mybir.dt.int32, mybir.dt.int16
BF16 = mybir.dt.bfloat16
HID, H, DH, INC, OUTC = 64, 4, 16, 128, 16
LMAX = 3
KVROW = LMAX * 2 * HID  # 384 elements per node row in kv table
HALF = 25088            # row split so int16 indices stay positive
C_MAX = 8               # chunks per compute superchunk
GW = 8                  # chunks per gather window (<=1024 idx per dma_gather)


def _wrap16(arr_cm):
    """[tot_ch, 128] int -> [128, tot_ch*8] int16 ucode idx layout."""
    tc_, _ = arr_cm.shape
    a = arr_cm.reshape(tc_, 8, 16).transpose(2, 0, 1).reshape(16, tc_ * 8)
    out = np.zeros((128, tc_ * 8), np.int16)
    for r in range(8):
        out[r * 16:(r + 1) * 16] = a
    return out


def make_cfg(n_nodes, edge_index, n_cores=8):
    tpc = math.ceil(n_nodes / n_cores / 128)
    npad = n_cores * tpc * 128
    nsh = tpc * 128
    half = HALF if npad + 128 > 32000 else npad + 128

    src_e = np.asarray(edge_index[0], dtype=np.int64)
    dst_e = np.asarray(edge_index[1], dtype=np.int64)
    loop = np.arange(n_nodes, dtype=np.int64)
    src = np.concatenate([src_e, loop])
    dst = np.concatenate([dst_e, loop])

    deg = np.bincount(dst, minlength=n_nodes).astype(np.float32)
    dinv = np.where(deg > 0, 1.0 / np.sqrt(np.maximum(deg, 1.0)), 0.0).astype(np.float32)

    # ---- node -> (core, slot) snake deal by degree --------------------
    order = np.argsort(-deg, kind="stable")  # high degree first
    slot_of = np.full(npad, -1, np.int64)
    core_nodes = [[] for _ in range(n_cores)]
    for r in range(0, n_nodes, n_cores):
        blk = order[r:r + n_cores]
        cs = range(n_cores) if (r // n_cores) % 2 == 0 else range(n_cores - 1, -1, -1)
        for c, nd in zip(cs, blk):
            core_nodes[c].append(nd)
    for c in range(n_cores):
        cn = np.asarray(core_nodes[c], np.int64)
        slot_of[cn] = c * nsh + np.arange(len(cn))
        core_nodes[c] = cn
    # dinv in slot order (pad slots -> 0)
    dinv_slot = np.zeros(npad, np.float32)
    for c in range(n_cores):
        dinv_slot[c * nsh:c * nsh + len(core_nodes[c])] = dinv[core_nodes[c]]

    # chunk path uses only the original edges; the added self-loops are
    # handled by the batched self path (own k/v rows, no gather)
    src_sl = slot_of[src_e]
    dst_sl = slot_of[dst_e]

    # bias correction: corr[n] = dinv[n] * sum_{e->n} dinv[src]
    ssum = np.zeros(n_nodes, np.float64)
    np.add.at(ssum, dst, dinv[src].astype(np.float64))
    corr = dinv * ssum.astype(np.float32)
    corr_slot = np.zeros(npad, np.float32)
    for c in range(n_cores):
        corr_slot[c * nsh:c * nsh + len(core_nodes[c])] = corr[core_nodes[c]]

    key = dst_sl * 2 + (src_sl >= half)
    order_e = np.argsort(key, kind="stable")
    src_s, dst_s = src_sl[order_e], dst_sl[order_e]

    cnt_lo = np.zeros((n_cores, tpc), np.int64)
    cnt_hi = np.zeros((n_cores, tpc), np.int64)
    core_of = dst_s // nsh
    ltile = (dst_s % nsh) // 128
    is_hi = src_s >= half
    np.add.at(cnt_lo, (core_of[~is_hi], ltile[~is_hi]), 1)
    np.add.at(cnt_hi, (core_of[is_hi], ltile[is_hi]), 1)
    kt_lo = np.maximum(np.ceil(cnt_lo.max(axis=0) / 128).astype(np.int64), 1)
    kt_hi = np.ceil(cnt_hi.max(axis=0) / 128).astype(np.int64)
    kt = kt_lo + kt_hi
    tot_ch = int(kt.sum())
    chunk_base = np.concatenate([[0], np.cumsum(kt)]).astype(np.int64)

    kvidx_cm = np.zeros((n_cores, tot_ch, 128), np.int64)  # pad -> row 0 (masked)
    dloc_cm = np.full((n_cores, tot_ch, 128), 128, np.int64)
    for c in range(n_cores):
        lo = np.searchsorted(dst_s, c * nsh)
        hi_ = np.searchsorted(dst_s, (c + 1) * nsh)
        sseg_c = src_s[lo:hi_]
        dseg_c = dst_s[lo:hi_]
        for t in range(tpc):
            g0 = c * nsh + t * 128
            l0 = np.searchsorted(dseg_c, g0)
            l1 = np.searchsorted(dseg_c, g0 + 128)
            if l1 == l0:
                continue
            sseg = sseg_c[l0:l1]
            dseg = dseg_c[l0:l1]
            hseg = sseg >= half
            for half_i, mask, base_ch in (
                (0, ~hseg, chunk_base[t]),
                (1, hseg, chunk_base[t] + kt_lo[t]),
            ):
                sv = sseg[mask]
                dv = dseg[mask]
                n = len(sv)
                if n == 0:
                    continue
                ch = base_ch + np.arange(n) // 128
                lane = np.arange(n) % 128
                kvidx_cm[c, ch, lane] = sv - (half if half_i else 0)
                dloc_cm[c, ch, lane] = dv - g0

    # gather windows (same enumeration as edge_layer) + per-core valid counts
    scs_all = []  # (tile, chunk_base+offset, cc) in issue order
    for t in range(tpc):
        for seg_o, seg_n in ((0, kt_lo[t]), (kt_lo[t], kt_hi[t])):
            o = 0
            while o < seg_n:
                w = min(GW, seg_n - o)
                scs_all.append((t, int(chunk_base[t]) + seg_o + o, w))
                o += w
    n_g = len(scs_all)

    return dict(
        n_cores=n_cores, tpc=tpc, npad=npad, nsh=nsh, n_nodes=n_nodes,
        half=half,
        kt=[int(k) for k in kt], kt_lo=[int(k) for k in kt_lo],
        kt_hi=[int(k) for k in kt_hi],
        tot_ch=tot_ch, chunk_base=[int(b) for b in chunk_base],
        n_g=n_g,
        kvidx=np.stack([_wrap16(kvidx_cm[c].astype(np.int16)) for c in range(n_cores)]),
        dlocT=np.ascontiguousarray(
            dloc_cm.astype(np.int32).transpose(0, 2, 1)),  # [c, 128, tot_ch]
        dinv_slot=dinv_slot, corr_slot=corr_slot,
        core_nodes=core_nodes,
    )


def prep_inputs(cfg, x, W_lin, b_lin, Wq, bq, Wk, bk, Wv, bv, W_out, b_out):
    ncore, npad, tpc, nsh = cfg["n_cores"], cfg["npad"], cfg["tpc"], cfg["nsh"]
    x = np.asarray(x, np.float32)
    bv = np.asarray(bv, np.float32)
    import ml_dtypes
    iota = np.broadcast_to(np.arange(128, dtype=np.float32), (128, 128))
    base = {
        "wlin": np.asarray(W_lin, np.float32),
        "wq": np.asarray(Wq, np.float32),
        "wk": np.asarray(Wk, np.float32),
        "wv": np.asarray(Wv, np.float32),
        "wout": np.asarray(W_out, np.float32),
        "blin_col": np.asarray(b_lin, np.float32).reshape(HID, 1),
        "bq_r": np.asarray(bq, np.float32).reshape(1, HID),
        "bout_r": np.asarray(b_out, np.float32).reshape(1, OUTC),
        "iota_bf": iota.astype(ml_dtypes.bfloat16),  # values 0..127 exact in bf16
    }
    in_maps = []
    for c in range(ncore):
        m = dict(base)
        cn = cfg["core_nodes"][c]
        xTL = np.zeros((INC, nsh), np.float32)
        xTL[:, :len(cn)] = x[cn].T
        m["xTL"] = xTL
        dsl = cfg["dinv_slot"][c * nsh:(c + 1) * nsh]
        m["dinvL"] = np.ascontiguousarray(dsl.reshape(tpc, 128).T)  # [128, tpc]
        csl = cfg["corr_slot"][c * nsh:(c + 1) * nsh]
        # biasT[p, t*64+j] = corr[slot t*128+p] * bv[j]
        bt = csl.reshape(tpc, 128)[:, :, None] * bv[None, None, :]  # [tpc,128,64]
        m["biasT"] = np.ascontiguousarray(
            bt.transpose(1, 0, 2).reshape(128, tpc * HID)).astype(np.float32)
        m["kvidx"] = cfg["kvidx"][c]
        # dloc as bf16 (values 0..128 exact)
        m["dlocT"] = cfg["dlocT"][c].astype(ml_dtypes.bfloat16)
        in_maps.append(m)
    return in_maps


def build_kernel(cfg):
    ncore, tpc, npad, nsh = cfg["n_cores"], cfg["tpc"], cfg["npad"], cfg["nsh"]
    kt, kt_lo, kt_hi = cfg["kt"], cfg["kt_lo"], cfg["kt_hi"]
    tot_ch, chunk_base, half = cfg["tot_ch"], cfg["chunk_base"], cfg["half"]

    # AllGather quarters: tile group boundaries
    qb = [0, tpc // 4, tpc // 2, 3 * tpc // 4, tpc]
    rows_h = [(qb[i + 1] - qb[i]) * 128 for i in range(4)]
    NQ = 4

    nc = bacc.Bacc("TRN2", target_bir_lowering=False, debug=False,
                   num_devices=ncore, num_swdge_queues=2)

    xTL = nc.dram_tensor("xTL", [INC, nsh], F32, kind="ExternalInput")
    wlin = nc.dram_tensor("wlin", [INC, HID], F32, kind="ExternalInput")
    wq = nc.dram_tensor("wq", [HID, HID], F32, kind="ExternalInput")
    wk = nc.dram_tensor("wk", [HID, HID], F32, kind="ExternalInput")
    wv = nc.dram_tensor("wv", [HID, HID], F32, kind="ExternalInput")
    wout = nc.dram_tensor("wout", [HID, OUTC], F32, kind="ExternalInput")
    blin_col = nc.dram_tensor("blin_col", [HID, 1], F32, kind="ExternalInput")
    bq_r = nc.dram_tensor("bq_r", [1, HID], F32, kind="ExternalInput")
    bout_r = nc.dram_tensor("bout_r", [1, OUTC], F32, kind="ExternalInput")
    iota_bf = nc.dram_tensor("iota_bf", [128, 128], BF16, kind="ExternalInput")
    dinvL_d = nc.dram_tensor("dinvL", [128, tpc], F32, kind="ExternalInput")
    biasT_d = nc.dram_tensor("biasT", [128, tpc * HID], F32, kind="ExternalInput")
    kvidx_d = nc.dram_tensor("kvidx", [128, tot_ch * 8], I16, kind="ExternalInput")
    dlocT_d = nc.dram_tensor("dlocT", [128, tot_ch], BF16, kind="ExternalInput")
    y = nc.dram_tensor("y", [nsh, OUTC], F32, kind="ExternalOutput")

    with tile.TileContext(nc) as tc:
        import contextlib
        ctx = contextlib.ExitStack()
        with ctx:
            cpool = ctx.enter_context(tc.tile_pool(name="const", bufs=1))
            dram = ctx.enter_context(tc.tile_pool(name="dram", bufs=1, space="DRAM"))

            kvtab = dram.tile([npad + 128, KVROW], BF16, name="kvtab")
            kvown = dram.tile([nsh, KVROW], BF16, name="kvown")
            # per (slice, quarter) collective buffers
            agin = [[dram.tile([rows_h[h], 128], BF16, name=f"agin{s}_{h}")
                     for h in range(NQ)] for s in (0, 1, 2)]
            agout = [[dram.tile([ncore * rows_h[h], 128], BF16,
                                name=f"agout{s}_{h}", addr_space="Shared")
                      for h in range(NQ)] for s in (0, 1, 2)]

            def load_const(dt_, shape, src_ap, name):
                t_ = cpool.tile(shape, dt_, name=name)
                nc.sync.dma_start(t_[:], src_ap)
                return t_

            wlin_s = load_const(F32, [INC, HID], wlin[:], "wlin_s")
            wq_s = load_const(F32, [HID, HID], wq[:], "wq_s")
            wk_s = load_const(F32, [HID, HID], wk[:], "wk_s")
            wv_s = load_const(F32, [HID, HID], wv[:], "wv_s")
            wout_s = load_const(F32, [HID, OUTC], wout[:], "wout_s")
            blin_s = load_const(F32, [HID, 1], blin_col[:], "blin_s")
            bq_s = load_const(F32, [1, HID], bq_r[:], "bq_s")
            bout_s = load_const(F32, [1, OUTC], bout_r[:], "bout_s")
            iota_s = load_const(BF16, [128, 128], iota_bf[:], "iota_s")
            dinvL_s = load_const(F32, [128, tpc], dinvL_d[:], "dinvL_s")
            biasT_s = load_const(F32, [128, tpc * HID], biasT_d[:], "biasT_s")
            iden = cpool.tile([128, 128], F32, name="iden")
            make_identity(nc, iden[:])
            iden_bf = cpool.tile([128, 128], BF16, name="iden_bf")
            nc.vector.tensor_copy(iden_bf[:], iden[:])
            ones_r = cpool.tile([1, 128], F32, name="ones_r")
            nc.vector.memset(ones_r[:], 1.0)
            # SBUF-resident q tiles for layers 2 and 3
            qstore = [cpool.tile([128, tpc * HID], BF16, name=f"qstore{i}")
                      for i in (0, 1)]

            sb_xt = ctx.enter_context(tc.tile_pool(name="sb_xt", bufs=3))
            sb_ht = ctx.enter_context(tc.tile_pool(name="sb_ht", bufs=3))
            sb_kv = ctx.enter_context(tc.tile_pool(name="sb_kv", bufs=3))
            sb_idx = ctx.enter_context(tc.tile_pool(name="sb_idx", bufs=8))
            sb_oh = ctx.enter_context(tc.tile_pool(name="sb_oh", bufs=6))
            sb_g = ctx.enter_context(tc.tile_pool(name="sb_g", bufs=5))
            sb_ve = ctx.enter_context(tc.tile_pool(name="sb_ve", bufs=5))
            sb_sm = ctx.enter_context(tc.tile_pool(name="sb_sm", bufs=6))
            sb_out = ctx.enter_context(tc.tile_pool(name="sb_out", bufs=3))
            sb_self = ctx.enter_context(tc.tile_pool(name="sb_self", bufs=2))
            sb_self2 = ctx.enter_context(tc.tile_pool(name="sb_self2", bufs=2))
            ps_big = ctx.enter_context(tc.tile_pool(name="ps_big", bufs=1, space="PSUM"))
            ps_kv = ctx.enter_context(tc.tile_pool(name="ps_kv", bufs=2, space="PSUM"))
            ps_out = ctx.enter_context(tc.tile_pool(name="ps_out", bufs=2, space="PSUM"))
            ps_qe = ctx.enter_context(tc.tile_pool(name="ps_qe", bufs=2, space="PSUM"))
            ps_oht = ctx.enter_context(tc.tile_pool(name="ps_oht", bufs=1, space="PSUM"))

            AF, ALU = mybir.ActivationFunctionType, mybir.AluOpType
            AX = mybir.AxisListType
            qctr = [0]

            def next_q():
                qctr[0] += 1
                return qctr[0] % 2

            def agslice(s, t):
                """(quarter, row0) for tile t in the slice-s AG input."""
                for h in range(NQ):
                    if t < qb[h + 1]:
                        return h, (t - qb[h]) * 128
                raise AssertionError(t)

            # ---- per-own-tile k/v (+q) build from hts [HID, 128] -------
            def own_kv_build(s, t, hts, hcols=0):
                kvp = ps_kv.tile([128, 2 * HID], F32, name="kvp", space="PSUM")
                nc.tensor.matmul(kvp[:, :HID], lhsT=hts[:, hcols:hcols + 128],
                                 rhs=wk_s[:], start=True, stop=True)
                nc.tensor.matmul(kvp[:, HID:], lhsT=hts[:, hcols:hcols + 128],
                                 rhs=wv_s[:], start=True, stop=True)
                kvsb = sb_kv.tile([128, 128], BF16, name="kvsb")
                nc.scalar.copy(kvsb[:, :HID], kvp[:, :HID])
                nc.scalar.mul(kvsb[:, HID:], kvp[:, HID:],
                              dinvL_s[:, t:t + 1])
                h_, r0 = agslice(s, t)
                nc.sync.dma_start(agin[s][h_][r0:r0 + 128, :], kvsb[:])
                nc.sync.dma_start(
                    kvown[t * 128:(t + 1) * 128, s * 128:(s + 1) * 128], kvsb[:])

            def own_q_build(qi, t, hts):
                qp = ps_kv.tile([128, 2 * HID], F32, name="kvp", space="PSUM")
                nc.tensor.matmul(qp[:, :HID], lhsT=hts[:, :128], rhs=wq_s[:],
                                 start=True, stop=False)
                nc.tensor.matmul(qp[:, :HID], lhsT=ones_r[:], rhs=bq_s[:],
                                 start=False, stop=True)
                nc.scalar.copy(qstore[qi][:, t * HID:(t + 1) * HID], qp[:, :HID])

            def fire_ag(s, h_):
                nc.gpsimd.collective_compute(
                    "AllGather", mybir.AluOpType.bypass,
                    replica_groups=[list(range(ncore))],
                    ins=[agin[s][h_].opt()], outs=[agout[s][h_].opt()])

            def copy_ag(s, h_):
                # copy into kvtab columns [s*128:(s+1)*128], rows by core block
                rh = rows_h[h_]
                off = qb[h_] * 128
                src = agout[s][h_][:].rearrange("(c r) d -> c r d", c=ncore)
                dst = kvtab[0:npad, s * 128:(s + 1) * 128].rearrange(
                    "(c r) d -> c r d", c=ncore, r=nsh)[:, off:off + rh, :]
                nc.sync.dma_start(dst, src)

            # ================= slice 0 (h = x @ W_lin + b) ==============
            def build_slice0():
                ng = nsh // 512  # 12 full groups of 512
                rem = (nsh - ng * 512) // 128
                for g in range(ng + 1):
                    nb = 4 if g < ng else rem
                    if nb == 0:
                        continue
                    w = nb * 128
                    xt_t = sb_xt.tile([INC, 512], F32, name="xt_t")
                    nc.sync.dma_start(xt_t[:, :w], xTL[:, g * 512:g * 512 + w])
                    htp = ps_big.tile([HID, 512], F32, name="htp", space="PSUM")
                    for b in range(nb):
                        nc.tensor.matmul(htp[:, b * 128:(b + 1) * 128], lhsT=wlin_s[:],
                                         rhs=xt_t[:, b * 128:(b + 1) * 128],
                                         start=True, stop=True)
                    hts = sb_ht.tile([HID, 512], F32, name="hts")
                    nc.vector.tensor_scalar(out=hts[:, :w], in0=htp[:, :w],
                                            scalar1=blin_s[:], scalar2=None,
                                            op0=ALU.add)
                    for b in range(nb):
                        t_ = g * 4 + b
                        own_kv_build(0, t_, hts, hcols=b * 128)
                        if t_ + 1 in qb[1:4]:
                            h_ = qb.index(t_ + 1) - 1
                            fire_ag(0, h_)
                            copy_ag(0, h_)

            HT = (tpc + 1) // 2  # tiles per self-batch half
            SELF_T = [(0, HT), (HT, tpc - HT)]

            def self_msgs(ell, hf):
                """Batched self-loop messages for tiles [t0, t0+nb).

                Returns (tile, stride, off): per-tile message is
                tile[:, (t-t0)*stride + off : ... + HID] (bf16, already
                premultiplied by dinv[own])."""
                t0, nb = SELF_T[hf]
                if ell == 1:
                    sv1 = sb_self2.tile([128, HT * HID], BF16, name="sv1")
                    nc.sync.dma_start(
                        sv1[:, :nb * HID].rearrange("p (b d) -> p b d", b=nb),
                        kvown[:].rearrange("(b p) e -> p b e", p=128)
                        [:, t0:t0 + nb, HID:2 * HID])
                    return sv1, HID, 0
                skv = sb_self.tile([128, HT * LMAX * 128], BF16, name="skv")
                nc.sync.dma_start(
                    skv[:, :nb * ell * 128].rearrange("p (b e) -> p b e", b=nb),
                    kvown[:].rearrange("(b p) e -> p b e", p=128)
                    [:, t0:t0 + nb, :ell * 128])
                kvvS = skv[:, :nb * ell * 128].rearrange(
                    "p (b l s h d) -> p b l s h d", b=nb, l=ell, s=2, h=H, d=DH)
                qbv = qstore[ell % 2][:, t0 * HID:(t0 + nb) * HID].rearrange(
                    "p (b u h d) -> p b u h d", b=nb, u=1, h=H).to_broadcast(
                    [128, nb, ell, H, DH])
                qks = sb_self2.tile([128, HT * LMAX * HID], BF16, name="qks")
                qksv = qks[:, :nb * ell * HID].rearrange(
                    "p (b l h d) -> p b l h d", b=nb, l=ell, h=H, d=DH)
                nc.vector.tensor_tensor(qksv, qbv, kvvS[:, :, :, 0, :, :], ALU.mult)
                ssS = sb_sm.tile([128, HT * LMAX * H], F32, name="ssS")
                nc.vector.reduce_sum(
                    ssS[:, :nb * ell * H].rearrange("p (b l h) -> p b l h",
                                                    b=nb, l=ell, h=H),
                    qksv, axis=AX.X)
                eS = sb_sm.tile([128, HT * LMAX * H], BF16, name="eS")
                nc.scalar.activation(eS[:, :nb * ell * H], ssS[:, :nb * ell * H],
                                     AF.Exp, scale=1.0 / math.sqrt(DH))
                denS = sb_sm.tile([128, HT * H], F32, name="denS")
                nc.vector.reduce_sum(
                    denS[:, :nb * H].rearrange("p (b h) -> p b h", b=nb),
                    eS[:, :nb * ell * H].rearrange("p (b l h) -> p b h l",
                                                   b=nb, l=ell, h=H),
                    axis=AX.X)
                rdenS = sb_sm.tile([128, HT * H], BF16, name="rdenS")
                with nc.allow_low_precision(reason="bf16 softmax weights"):
                    nc.vector.reciprocal(rdenS[:, :nb * H], denS[:, :nb * H])
                attS = sb_sm.tile([128, HT * LMAX * H], BF16, name="attS")
                atv = attS[:, :nb * ell * H].rearrange("p (b l h) -> p b l h",
                                                       b=nb, l=ell, h=H)
                nc.vector.tensor_tensor(
                    atv,
                    eS[:, :nb * ell * H].rearrange("p (b l h) -> p b l h",
                                                   b=nb, l=ell, h=H),
                    rdenS[:, :nb * H].rearrange("p (b u h) -> p b u h", b=nb, u=1)
                    .to_broadcast([128, nb, ell, H]),
                    ALU.mult)
                msgS = sb_self2.tile([128, HT * HID], BF16, name="msgS")
                wvS = sb_self2.tile([128, HT * HID], BF16, name="wvS")
                mS = msgS[:, :nb * HID].rearrange("p (b h d) -> p b h d", b=nb, h=H)
                wS = wvS[:, :nb * HID].rearrange("p (b h d) -> p b h d", b=nb, h=H)
                for l in range(ell):
                    nc.vector.tensor_tensor(
                        mS if l == 0 else wS,
                        atv[:, :, l, :].to_broadcast([128, nb, H, DH]),
                        kvvS[:, :, l, 1, :, :], ALU.mult)
                    if l > 0:
                        nc.vector.tensor_add(msgS[:, :nb * HID],
                                             msgS[:, :nb * HID],
                                             wvS[:, :nb * HID])
                return msgS, HID, 0

            def attn_block(ell, t, kvg, oo, cc, oh):
                L = ell
                qtile = qstore[ell % 2][:, t * HID:(t + 1) * HID]
                qe = ps_qe.tile([128, C_MAX * HID], F32, name="qe",
                                space="PSUM")
                for k in range(cc):
                    ohtp = ps_oht.tile([128, 128], BF16, name="ohtp",
                                       space="PSUM")
                    nc.tensor.transpose(ohtp[:],
                                        in_=oh[:, k * 128:(k + 1) * 128],
                                        identity=iden_bf[:])
                    ohts = sb_oh.tile([128, 128], BF16, name="ohts")
                    nc.scalar.copy(ohts[:], ohtp[:])
                    nc.tensor.matmul(qe[:, k * HID:(k + 1) * HID],
                                     lhsT=ohts[:], rhs=qtile,
                                     start=True, stop=True)

                qeb = sb_ve.tile([128, C_MAX * HID], BF16, name="qeb")
                nc.scalar.copy(qeb[:, :cc * HID], qe[:, :cc * HID])
                kvv = kvg[:, oo * L * 128:(oo + cc) * L * 128].rearrange(
                    "p (c l s h d) -> p c l s h d", c=cc, l=L, s=2, h=H, d=DH)
                qv = qeb[:, :cc * HID].rearrange("p (c h d) -> p c h d",
                                                 h=H, d=DH)
                satt = sb_sm.tile([128, C_MAX * H * LMAX], F32, name="satt")
                sv = satt[:, :cc * H * L].rearrange("p (c h l) -> p c h l",
                                                    h=H, l=L)
                qk = sb_ve.tile([128, C_MAX * HID], BF16, name="qk")
                qkv = qk[:, :cc * HID].rearrange("p (c h d) -> p c h d",
                                                 h=H, d=DH)
                for l in range(L):
                    nc.vector.tensor_tensor(qkv, qv, kvv[:, :, l, 0, :, :],
                                            ALU.mult)
                    nc.vector.reduce_sum(sv[:, :, :, l], qkv, axis=AX.X)
                eatt = sb_sm.tile([128, C_MAX * H * LMAX], BF16, name="eatt")
                nc.scalar.activation(eatt[:, :cc * H * L], satt[:, :cc * H * L],
                                     AF.Exp, scale=1.0 / math.sqrt(DH))
                den = sb_sm.tile([128, C_MAX * H], F32, name="den")
                nc.vector.reduce_sum(
                    den[:, :cc * H].rearrange("p (c h) -> p c h", h=H),
                    eatt[:, :cc * H * L].rearrange("p (c h l) -> p c h l",
                                                   h=H, l=L),
                    axis=AX.X)
                rden = sb_sm.tile([128, C_MAX * H], BF16, name="rden")
                with nc.allow_low_precision(
                        reason="bf16 softmax weights; rel-err gate 2e-2"):
                    nc.vector.reciprocal(rden[:, :cc * H], den[:, :cc * H])
                att = sb_sm.tile([128, C_MAX * H * LMAX], BF16, name="att")
                av = att[:, :cc * H * L].rearrange("p (c h l) -> p c h l",
                                                   h=H, l=L)
                nc.vector.tensor_tensor(
                    av,
                    eatt[:, :cc * H * L].rearrange("p (c h l) -> p c h l",
                                                   h=H, l=L),
                    rden[:, :cc * H].rearrange("p (c h u) -> p c h u",
                                               h=H, u=1)
                    .to_broadcast([128, cc, H, L]),
                    ALU.mult)
                msg = sb_ve.tile([128, C_MAX * HID], BF16, name="msg")
                wvt = sb_ve.tile([128, C_MAX * HID], BF16, name="wvt")
                mv = msg[:, :cc * HID].rearrange("p (c h d) -> p c h d",
                                                 h=H, d=DH)
                wvv = wvt[:, :cc * HID].rearrange("p (c h d) -> p c h d",
                                                  h=H, d=DH)
                for l in range(L):
                    nc.vector.tensor_tensor(
                        mv if l == 0 else wvv,
                        av[:, :, :, l].to_broadcast([128, cc, H, DH]),
                        kvv[:, :, l, 1, :, :], ALU.mult)
                    if l > 0:
                        nc.vector.tensor_add(msg[:, :cc * HID],
                                             msg[:, :cc * HID],
                                             wvt[:, :cc * HID])
                return lambda k: msg[:, k * HID:(k + 1) * HID]

            # ================= edge pass =================
            def edge_layer(ell):
                L = ell
                selfb = [self_msgs(ell, 0), self_msgs(ell, 1)]
                for t in range(tpc):
                    stile, sstride, soff = selfb[0 if t < HT else 1]
                    st0 = 0 if t < HT else HT
                    po = ps_out.tile([128, HID], F32, name="po", space="PSUM")
                    n_ch = kt[t]
                    base = chunk_base[t]
                    done = 0
                    wins = []
                    for seg_o, seg_n in ((0, kt_lo[t]), (kt_lo[t], kt_hi[t])):
                        o = 0
                        while o < seg_n:
                            w = min(GW, seg_n - o)
                            wins.append((seg_o + o, w))
                            o += w
                    for (wo, gw) in wins:
                        rb = 0 if wo < kt_lo[t] else half
                        wcb = base + wo
                        kvi = sb_idx.tile([128, GW * 8], I16, name="kvi")
                        nc.sync.dma_start(kvi[:, :gw * 8],
                                          kvidx_d[:, wcb * 8:(wcb + gw) * 8])
                        if ell == 1:
                            gt = sb_g.tile([128, GW * LMAX * 128], BF16, name="kvg")
                            nc.gpsimd.dma_gather(
                                out_ap=gt[:, :gw * 128].rearrange(
                                    "p (n d) -> p n d", d=128),
                                in_ap=kvtab[rb:, 0:128],
                                idxs_ap=kvi[:, :gw * 8],
                                num_idxs=gw * 128, num_idxs_reg=gw * 128,
                                elem_size=128, elem_step=KVROW,
                                queue_num=next_q())
                        else:
                            gt = sb_g.tile([128, GW * LMAX * 128], BF16, name="kvg")
                            nc.gpsimd.dma_gather(
                                out_ap=gt[:, :gw * L * 128].rearrange(
                                    "p (n d) -> p n d", d=L * 128),
                                in_ap=kvtab[rb:, :L * 128],
                                idxs_ap=kvi[:, :gw * 8],
                                num_idxs=gw * 128, num_idxs_reg=gw * 128,
                                elem_size=L * 128, elem_step=KVROW,
                                queue_num=next_q())
                        oo = 0
                        while oo < gw:
                            cc = min(C_MAX, gw - oo)
                            cb = wcb + oo
                            dli = sb_idx.tile([128, C_MAX], BF16, name="dli")
                            nc.sync.dma_start(dli[:, :cc], dlocT_d[:, cb:cb + cc])
                            oh = sb_oh.tile([128, C_MAX * 128], BF16, name="oh")
                            nc.vector.tensor_tensor(
                                oh[:, :cc * 128].rearrange("p (c i) -> p c i",
                                                           c=cc, i=128),
                                dli[:, :cc].rearrange("p (c u) -> p c u", c=cc, u=1)
                                .to_broadcast([128, cc, 128]),
                                iota_s[:].rearrange("p (u i) -> p u i", u=1, i=128)
                                .to_broadcast([128, cc, 128]),
                                ALU.is_equal)
                            if ell == 1:
                                vgv = gt[:, oo * 128:(oo + cc) * 128].rearrange(
                                    "p (c s d) -> p c s d", s=2, d=HID)
                                mslice = lambda k, vgv=vgv: vgv[:, k, 1, :]
                            else:
                                mslice = attn_block(ell, t, gt, oo, cc, oh)
                            for k in range(cc):
                                nc.tensor.matmul(po[:],
                                                 lhsT=oh[:, k * 128:(k + 1) * 128],
                                                 rhs=mslice(k),
                                                 start=(done + k == 0),
                                                 stop=(done + k == n_ch - 1))
                            done += cc
                            oo += cc

                    # out = (po + self) * dinv[dst] + corr[dst] * bv  (2 fused ops)
                    so = (t - st0) * sstride + soff
                    ob0 = sb_out.tile([128, HID], F32, name="ob0")
                    nc.vector.scalar_tensor_tensor(
                        out=ob0[:], in0=stile[:, so:so + HID],
                        scalar=dinvL_s[:, t:t + 1],
                        in1=biasT_s[:, t * HID:(t + 1) * HID],
                        op0=ALU.mult, op1=ALU.add)
                    outsb = sb_out.tile([128, HID], F32, name="outsb")
                    nc.vector.scalar_tensor_tensor(
                        out=outsb[:], in0=po[:], scalar=dinvL_s[:, t:t + 1],
                        in1=ob0[:], op0=ALU.mult, op1=ALU.add)
                    if ell < 3:
                        # shared transpose for q-build and kv-build
                        htp = ps_big.tile([HID, 512], F32, name="htp", space="PSUM")
                        nc.tensor.transpose(htp[:, :128], in_=outsb[:], identity=iden[:])
                        hts = sb_ht.tile([HID, 512], F32, name="hts")
                        nc.scalar.copy(hts[:, :128], htp[:, :128])
                        own_q_build((ell + 1) % 2, t, hts)
                        own_kv_build(ell, t, hts)
                        if t + 1 in qb[1:]:
                            h_ = qb.index(t + 1) - 1
                            fire_ag(ell, h_)
                            copy_ag(ell, h_)
                    else:
                        final_tile(outsb, t)

            def final_tile(outsb, t):
                htp = ps_big.tile([HID, 512], F32, name="htp", space="PSUM")
                nc.tensor.transpose(htp[:, :128], in_=outsb[:], identity=iden[:])
                hts = sb_ht.tile([HID, 512], F32, name="hts")
                nc.scalar.copy(hts[:, :128], htp[:, :128])
                yp = ps_kv.tile([128, 2 * HID], F32, name="kvp", space="PSUM")
                nc.tensor.matmul(yp[:, :OUTC], lhsT=hts[:, :128], rhs=wout_s[:],
                                 start=True, stop=False)
                nc.tensor.matmul(yp[:, :OUTC], lhsT=ones_r[:], rhs=bout_s[:],
                                 start=False, stop=True)
                ysb = sb_out.tile([128, OUTC], F32, name="ysb")
                nc.scalar.copy(ysb[:], yp[:, :OUTC])
                nc.sync.dma_start(y[t * 128:(t + 1) * 128, :], ysb[:])

            # ================= schedule =================
            build_slice0()
            fire_ag(0, 3)
            copy_ag(0, 3)
            edge_layer(1)
            edge_layer(2)
            edge_layer(3)

    nc.compile()
    return nc


def assemble_output(cfg, results):
    n = cfg["n_nodes"]
    out = np.zeros((n, OUTC), np.float32)
    for c in range(cfg["n_cores"]):
        cn = cfg["core_nodes"][c]
        out[cn] = results[c]["y"][:len(cn)]
    return out


# ======================= harness entry point =======================
LAST_EXEC_NS = [None]
LAST_RESULT = [None]


def kernel(**inputs):
    """Full (unsharded) inputs -> full [N, 16] float32 output."""
    from concourse.bass_utils import run_bass_kernel_spmd

    x = np.asarray(inputs["x"], np.float32)
    edge_index = np.asarray(inputs["edge_index"])
    cfg = make_cfg(x.shape[0], edge_index, n_cores=8)
    in_maps = prep_inputs(
        cfg, x,
        inputs["W_lin"], inputs["b_lin"],
        inputs["Wq"], inputs["bq"],
        inputs["Wk"], inputs["bk"],
        inputs["Wv"], inputs["bv"],
        inputs["W_out"], inputs["b_out"],
    )
    nc = build_kernel(cfg)
    res = run_bass_kernel_spmd(nc, in_maps, core_ids=list(range(cfg["n_cores"])))
    LAST_EXEC_NS[0] = res.exec_time_ns
    LAST_RESULT[0] = res
    return assemble_output(cfg, res.results)


# revision 4
# speedup vs baseline: 1.1630x; 1.1379x over previous
"""DNANet Bass kernel v3.

Changes vs v2:
- Node->slot remap: nodes degree-sorted and snake-dealt across cores so
  per-core edge counts (and per-tile chunk counts) match across cores;
  kvtab rows are in slot order.
- Each core builds k/v only for its OWN nodes (interleaved into the edge
  pass, sharing the q-build transpose), then the k/v table slice is
  AllGathered (in two halves, first half overlapped with the edge pass)
  and copied into the interleaved kvtab columns.  This removes the
  ~0.9ms all-nodes table rebuild between layers.
- bk dropped entirely (softmax over layers is invariant to the q.bk
  shift); bv folded into a host-precomputed rank-1 per-node correction
  (sum of attention weights is 1), applied with one fused
  scalar_tensor_tensor per tile.
- q tiles stay resident in SBUF (no DRAM round trip).
- Gather pad indices are negative (skipped by SWDGE) to trim Q7
  descriptor-generation time.
- onehot inputs in bf16 for 2x DVE rate; PSUM->SBUF casts moved to the
  (otherwise idle) scalar engine.
"""
import math
import numpy as np
import concourse.bacc as bacc
import concourse.mybir as mybir
import concourse.tile as tile
from concourse.masks import make_identity

F32, I32, I16 = mybir.dt.float32, mybir.dt.int32, mybir.dt.int16
BF16 = mybir.dt.bfloat16
HID, H, DH, INC, OUTC = 64, 4, 16, 128, 16
LMAX = 3
KVROW = LMAX * 2 * HID  # 384 elements per node row in kv table
HALF = 25088            # row split so int16 indices stay positive
C_MAX = 8               # chunks per compute superchunk
GW = 8                  # chunks per gather window (<=1024 idx per dma_gather)


def _wrap16(arr_cm):
    """[tot_ch, 128] int -> [128, tot_ch*8] int16 ucode idx layout."""
    tc_, _ = arr_cm.shape
    a = arr_cm.reshape(tc_, 8, 16).transpose(2, 0, 1).reshape(16, tc_ * 8)
    out = np.zeros((128, tc_ * 8), np.int16)
    for r in range(8):
        out[r * 16:(r + 1) * 16] = a
    return out


def make_cfg(n_nodes, edge_index, n_cores=8):
    tpc = math.ceil(n_nodes / n_cores / 128)
    npad = n_cores * tpc * 128
    nsh = tpc * 128
    half = HALF if npad + 128 > 32000 else npad + 128

    src_e = np.asarray(edge_index[0], dtype=np.int64)
    dst_e = np.asarray(edge_index[1], dtype=np.int64)
    loop = np.arange(n_nodes, dtype=np.int64)
    src = np.concatenate([src_e, loop])
    dst = np.concatenate([dst_e, loop])

    deg = np.bincount(dst, minlength=n_nodes).astype(np.float32)
    dinv = np.where(deg > 0, 1.0 / np.sqrt(np.maximum(deg, 1.0)), 0.0).astype(np.float32)

    # ---- node -> (core, slot) snake deal by degree --------------------
    order = np.argsort(-deg, kind="stable")  # high degree first
    slot_of = np.full(npad, -1, np.int64)
    core_nodes = [[] for _ in range(n_cores)]
    for r in range(0, n_nodes, n_cores):
        blk = order[r:r + n_cores]
        cs = range(n_cores) if (r // n_cores) % 2 == 0 else range(n_cores - 1, -1, -1)
        for c, nd in zip(cs, blk):
            core_nodes[c].append(nd)
    for c in range(n_cores):
        cn = np.asarray(core_nodes[c], np.int64)
        slot_of[cn] = c * nsh + np.arange(len(cn))
        core_nodes[c] = cn
    # dinv in slot order (pad slots -> 0)
    dinv_slot = np.zeros(npad, np.float32)
    for c in range(n_cores):
        dinv_slot[c * nsh:c * nsh + len(core_nodes[c])] = dinv[core_nodes[c]]

    # chunk path uses only the original edges; the added self-loops are
    # handled by the batched self path (own k/v rows, no gather)
    src_sl = slot_of[src_e]
    dst_sl = slot_of[dst_e]

    # bias correction: corr[n] = dinv[n] * sum_{e->n} dinv[src]
    ssum = np.zeros(n_nodes, np.float64)
    np.add.at(ssum, dst, dinv[src].astype(np.float64))
    corr = dinv * ssum.astype(np.float32)
    corr_slot = np.zeros(npad, np.float32)
    for c in range(n_cores):
        corr_slot[c * nsh:c * nsh + len(core_nodes[c])] = corr[core_nodes[c]]

    key = dst_sl * 2 + (src_sl >= half)
    order_e = np.argsort(key, kind="stable")
    src_s, dst_s = src_sl[order_e], dst_sl[order_e]

    cnt_lo = np.zeros((n_cores, tpc), np.int64)
    cnt_hi = np.zeros((n_cores, tpc), np.int64)
    core_of = dst_s // nsh
    ltile = (dst_s % nsh) // 128
    is_hi = src_s >= half
    np.add.at(cnt_lo, (core_of[~is_hi], ltile[~is_hi]), 1)
    np.add.at(cnt_hi, (core_of[is_hi], ltile[is_hi]), 1)
    kt_lo = np.maximum(np.ceil(cnt_lo.max(axis=0) / 128).astype(np.int64), 1)
    kt_hi = np.ceil(cnt_hi.max(axis=0) / 128).astype(np.int64)
    kt = kt_lo + kt_hi
    tot_ch = int(kt.sum())
    chunk_base = np.concatenate([[0], np.cumsum(kt)]).astype(np.int64)

    kvidx_cm = np.zeros((n_cores, tot_ch, 128), np.int64)  # pad -> row 0 (masked)
    dloc_cm = np.full((n_cores, tot_ch, 128), 128, np.int64)
    for c in range(n_cores):
        lo = np.searchsorted(dst_s, c * nsh)
        hi_ = np.searchsorted(dst_s, (c + 1) * nsh)
        sseg_c = src_s[lo:hi_]
        dseg_c = dst_s[lo:hi_]
        for t in range(tpc):
            g0 = c * nsh + t * 128
            l0 = np.searchsorted(dseg_c, g0)
            l1 = np.searchsorted(dseg_c, g0 + 128)
            if l1 == l0:
                continue
            sseg = sseg_c[l0:l1]
            dseg = dseg_c[l0:l1]
            hseg = sseg >= half
            for half_i, mask, base_ch in (
                (0, ~hseg, chunk_base[t]),
                (1, hseg, chunk_base[t] + kt_lo[t]),
            ):
                sv = sseg[mask]
                dv = dseg[mask]
                n = len(sv)
                if n == 0:
                    continue
                ch = base_ch + np.arange(n) // 128
                lane = np.arange(n) % 128
                kvidx_cm[c, ch, lane] = sv - (half if half_i else 0)
                dloc_cm[c, ch, lane] = dv - g0

    # gather windows (same enumeration as edge_layer) + per-core valid counts
    scs_all = []  # (tile, chunk_base+offset, cc) in issue order
    for t in range(tpc):
        for seg_o, seg_n in ((0, kt_lo[t]), (kt_lo[t], kt_hi[t])):
            o = 0
            while o < seg_n:
                w = min(GW, seg_n - o)
                scs_all.append((t, int(chunk_base[t]) + seg_o + o, w))
                o += w
    n_g = len(scs_all)

    return dict(
        n_cores=n_cores, tpc=tpc, npad=npad, nsh=nsh, n_nodes=n_nodes,
        half=half,
        kt=[int(k) for k in kt], kt_lo=[int(k) for k in kt_lo],
        kt_hi=[int(k) for k in kt_hi],
        tot_ch=tot_ch, chunk_base=[int(b) for b in chunk_base],
        n_g=n_g,
        kvidx=np.stack([_wrap16(kvidx_cm[c].astype(np.int16)) for c in range(n_cores)]),
        dlocT=np.ascontiguousarray(
            dloc_cm.astype(np.int32).transpose(0, 2, 1)),  # [c, 128, tot_ch]
        dinv_slot=dinv_slot, corr_slot=corr_slot,
        core_nodes=core_nodes,
    )


def prep_inputs(cfg, x, W_lin, b_lin, Wq, bq, Wk, bk, Wv, bv, W_out, b_out):
    ncore, npad, tpc, nsh = cfg["n_cores"], cfg["npad"], cfg["tpc"], cfg["nsh"]
    x = np.asarray(x, np.float32)
    bv = np.asarray(bv, np.float32)
    import ml_dtypes
    iota = np.broadcast_to(np.arange(128, dtype=np.float32), (128, 128))
    base = {
        "wlin": np.asarray(W_lin, np.float32),
        "wq": np.asarray(Wq, np.float32),
        "wk": np.asarray(Wk, np.float32),
        "wv": np.asarray(Wv, np.float32),
        "wout": np.asarray(W_out, np.float32),
        "blin_col": np.asarray(b_lin, np.float32).reshape(HID, 1),
        "bq_r": np.asarray(bq, np.float32).reshape(1, HID),
        "bout_r": np.asarray(b_out, np.float32).reshape(1, OUTC),
        "iota_bf": iota.astype(ml_dtypes.bfloat16),  # values 0..127 exact in bf16
    }
    in_maps = []
    for c in range(ncore):
        m = dict(base)
        cn = cfg["core_nodes"][c]
        xTL = np.zeros((INC, nsh), np.float32)
        xTL[:, :len(cn)] = x[cn].T
        m["xTL"] = xTL
        dsl = cfg["dinv_slot"][c * nsh:(c + 1) * nsh]
        m["dinvL"] = np.ascontiguousarray(dsl.reshape(tpc, 128).T)  # [128, tpc]
        csl = cfg["corr_slot"][c * nsh:(c + 1) * nsh]
        # biasT[p, t*64+j] = corr[slot t*128+p] * bv[j]
        bt = csl.reshape(tpc, 128)[:, :, None] * bv[None, None, :]  # [tpc,128,64]
        m["biasT"] = np.ascontiguousarray(
            bt.transpose(1, 0, 2).reshape(128, tpc * HID)).astype(np.float32)
        m["kvidx"] = cfg["kvidx"][c]
        # dloc as bf16 (values 0..128 exact)
        m["dlocT"] = cfg["dlocT"][c].astype(ml_dtypes.bfloat16)
        in_maps.append(m)
    return in_maps


def build_kernel(cfg):
    ncore, tpc, npad, nsh = cfg["n_cores"], cfg["tpc"], cfg["npad"], cfg["nsh"]
    kt, kt_lo, kt_hi = cfg["kt"], cfg["kt_lo"], cfg["kt_hi"]
    tot_ch, chunk_base, half = cfg["tot_ch"], cfg["chunk_base"], cfg["half"]

    # AllGather quarters: tile group boundaries
    qb = [0, 2 * tpc // 7, 4 * tpc // 7, 6 * tpc // 7, tpc]
    rows_h = [(qb[i + 1] - qb[i]) * 128 for i in range(4)]
    NQ = 4

    nc = bacc.Bacc("TRN2", target_bir_lowering=False, debug=False,
                   num_devices=ncore, num_swdge_queues=2)

    xTL = nc.dram_tensor("xTL", [INC, nsh], F32, kind="ExternalInput")
    wlin = nc.dram_tensor("wlin", [INC, HID], F32, kind="ExternalInput")
    wq = nc.dram_tensor("wq", [HID, HID], F32, kind="ExternalInput")
    wk = nc.dram_tensor("wk", [HID, HID], F32, kind="ExternalInput")
    wv = nc.dram_tensor("wv", [HID, HID], F32, kind="ExternalInput")
    wout = nc.dram_tensor("wout", [HID, OUTC], F32, kind="ExternalInput")
    blin_col = nc.dram_tensor("blin_col", [HID, 1], F32, kind="ExternalInput")
    bq_r = nc.dram_tensor("bq_r", [1, HID], F32, kind="ExternalInput")
    bout_r = nc.dram_tensor("bout_r", [1, OUTC], F32, kind="ExternalInput")
    iota_bf = nc.dram_tensor("iota_bf", [128, 128], BF16, kind="ExternalInput")
    dinvL_d = nc.dram_tensor("dinvL", [128, tpc], F32, kind="ExternalInput")
    biasT_d = nc.dram_tensor("biasT", [128, tpc * HID], F32, kind="ExternalInput")
    kvidx_d = nc.dram_tensor("kvidx", [128, tot_ch * 8], I16, kind="ExternalInput")
    dlocT_d = nc.dram_tensor("dlocT", [128, tot_ch], BF16, kind="ExternalInput")
    y = nc.dram_tensor("y", [nsh, OUTC], F32, kind="ExternalOutput")

    with tile.TileContext(nc) as tc:
        import contextlib
        ctx = contextlib.ExitStack()
        with ctx:
            cpool = ctx.enter_context(tc.tile_pool(name="const", bufs=1))
            dram = ctx.enter_context(tc.tile_pool(name="dram", bufs=1, space="DRAM"))

            kvtab = dram.tile([npad + 128, KVROW], BF16, name="kvtab")
            kvown = dram.tile([nsh, KVROW], BF16, name="kvown")
            # per (slice, quarter) collective buffers
            agin = [[dram.tile([rows_h[h], 128], BF16, name=f"agin{s}_{h}")
                     for h in range(NQ)] for s in (0, 1, 2)]
            agout = [[dram.tile([ncore * rows_h[h], 128], BF16,
                                name=f"agout{s}_{h}", addr_space="Shared")
                      for h in range(NQ)] for s in (0, 1, 2)]

            def load_const(dt_, shape, src_ap, name):
                t_ = cpool.tile(shape, dt_, name=name)
                nc.sync.dma_start(t_[:], src_ap)
                return t_

            wlin_s = load_const(F32, [INC, HID], wlin[:], "wlin_s")
            wq_s = load_const(F32, [HID, HID], wq[:], "wq_s")
            wk_s = load_const(F32, [HID, HID], wk[:], "wk_s")
            wv_s = load_const(F32, [HID, HID], wv[:], "wv_s")
            wout_s = load_const(F32, [HID, OUTC], wout[:], "wout_s")
            blin_s = load_const(F32, [HID, 1], blin_col[:], "blin_s")
            bq_s = load_const(F32, [1, HID], bq_r[:], "bq_s")
            bout_s = load_const(F32, [1, OUTC], bout_r[:], "bout_s")
            iota_s = load_const(BF16, [128, 128], iota_bf[:], "iota_s")
            dinvL_s = load_const(F32, [128, tpc], dinvL_d[:], "dinvL_s")
            biasT_s = load_const(F32, [128, tpc * HID], biasT_d[:], "biasT_s")
            iden = cpool.tile([128, 128], F32, name="iden")
            make_identity(nc, iden[:])
            iden_bf = cpool.tile([128, 128], BF16, name="iden_bf")
            nc.vector.tensor_copy(iden_bf[:], iden[:])
            ones_r = cpool.tile([1, 128], F32, name="ones_r")
            nc.vector.memset(ones_r[:], 1.0)
            # SBUF-resident q tiles for layers 2 and 3
            qstore = [cpool.tile([128, tpc * HID], BF16, name=f"qstore{i}")
                      for i in (0, 1)]

            sb_xt = ctx.enter_context(tc.tile_pool(name="sb_xt", bufs=3))
            sb_ht = ctx.enter_context(tc.tile_pool(name="sb_ht", bufs=3))
            sb_kv = ctx.enter_context(tc.tile_pool(name="sb_kv", bufs=3))
            sb_idx = ctx.enter_context(tc.tile_pool(name="sb_idx", bufs=8))
            sb_oh = ctx.enter_context(tc.tile_pool(name="sb_oh", bufs=6))
            sb_g = ctx.enter_context(tc.tile_pool(name="sb_g", bufs=5))
            sb_ve = ctx.enter_context(tc.tile_pool(name="sb_ve", bufs=5))
            sb_sm = ctx.enter_context(tc.tile_pool(name="sb_sm", bufs=6))
            sb_out = ctx.enter_context(tc.tile_pool(name="sb_out", bufs=3))
            sb_self = ctx.enter_context(tc.tile_pool(name="sb_self", bufs=2))
            sb_self2 = ctx.enter_context(tc.tile_pool(name="sb_self2", bufs=2))
            ps_big = ctx.enter_context(tc.tile_pool(name="ps_big", bufs=1, space="PSUM"))
            ps_kv = ctx.enter_context(tc.tile_pool(name="ps_kv", bufs=2, space="PSUM"))
            ps_out = ctx.enter_context(tc.tile_pool(name="ps_out", bufs=2, space="PSUM"))
            ps_qe = ctx.enter_context(tc.tile_pool(name="ps_qe", bufs=2, space="PSUM"))
            ps_oht = ctx.enter_context(tc.tile_pool(name="ps_oht", bufs=1, space="PSUM"))

            AF, ALU = mybir.ActivationFunctionType, mybir.AluOpType
            AX = mybir.AxisListType
            qctr = [0]

            def next_q():
                qctr[0] += 1
                return qctr[0] % 2

            def agslice(s, t):
                """(quarter, row0) for tile t in the slice-s AG input."""
                for h in range(NQ):
                    if t < qb[h + 1]:
                        return h, (t - qb[h]) * 128
                raise AssertionError(t)

            # ---- per-own-tile k/v (+q) build from hts [HID, 128] -------
            def own_kv_build(s, t, hts, hcols=0):
                kvp = ps_kv.tile([128, 2 * HID], F32, name="kvp", space="PSUM")
                nc.tensor.matmul(kvp[:, :HID], lhsT=hts[:, hcols:hcols + 128],
                                 rhs=wk_s[:], start=True, stop=True)
                nc.tensor.matmul(kvp[:, HID:], lhsT=hts[:, hcols:hcols + 128],
                                 rhs=wv_s[:], start=True, stop=True)
                kvsb = sb_kv.tile([128, 128], BF16, name="kvsb")
                nc.scalar.copy(kvsb[:, :HID], kvp[:, :HID])
                nc.scalar.mul(kvsb[:, HID:], kvp[:, HID:],
                              dinvL_s[:, t:t + 1])
                h_, r0 = agslice(s, t)
                nc.sync.dma_start(agin[s][h_][r0:r0 + 128, :], kvsb[:])
                nc.sync.dma_start(
                    kvown[t * 128:(t + 1) * 128, s * 128:(s + 1) * 128], kvsb[:])

            def own_q_build(qi, t, hts):
                qp = ps_kv.tile([128, 2 * HID], F32, name="kvp", space="PSUM")
                nc.tensor.matmul(qp[:, :HID], lhsT=hts[:, :128], rhs=wq_s[:],
                                 start=True, stop=False)
                nc.tensor.matmul(qp[:, :HID], lhsT=ones_r[:], rhs=bq_s[:],
                                 start=False, stop=True)
                nc.scalar.copy(qstore[qi][:, t * HID:(t + 1) * HID], qp[:, :HID])

            def fire_ag(s, h_):
                nc.gpsimd.collective_compute(
                    "AllGather", mybir.AluOpType.bypass,
                    replica_groups=[list(range(ncore))],
                    ins=[agin[s][h_].opt()], outs=[agout[s][h_].opt()])

            def copy_ag(s, h_):
                # copy into kvtab columns [s*128:(s+1)*128], rows by core block
                rh = rows_h[h_]
                off = qb[h_] * 128
                src = agout[s][h_][:].rearrange("(c r) d -> c r d", c=ncore)
                dst = kvtab[0:npad, s * 128:(s + 1) * 128].rearrange(
                    "(c r) d -> c r d", c=ncore, r=nsh)[:, off:off + rh, :]
                nc.sync.dma_start(dst, src)

            # ================= slice 0 (h = x @ W_lin + b) ==============
            def build_slice0():
                ng = nsh // 512  # 12 full groups of 512
                rem = (nsh - ng * 512) // 128
                for g in range(ng + 1):
                    nb = 4 if g < ng else rem
                    if nb == 0:
                        continue
                    w = nb * 128
                    xt_t = sb_xt.tile([INC, 512], F32, name="xt_t")
                    nc.sync.dma_start(xt_t[:, :w], xTL[:, g * 512:g * 512 + w])
                    htp = ps_big.tile([HID, 512], F32, name="htp", space="PSUM")
                    for b in range(nb):
                        nc.tensor.matmul(htp[:, b * 128:(b + 1) * 128], lhsT=wlin_s[:],
                                         rhs=xt_t[:, b * 128:(b + 1) * 128],
                                         start=True, stop=True)
                    hts = sb_ht.tile([HID, 512], F32, name="hts")
                    nc.vector.tensor_scalar(out=hts[:, :w], in0=htp[:, :w],
                                            scalar1=blin_s[:], scalar2=None,
                                            op0=ALU.add)
                    for b in range(nb):
                        t_ = g * 4 + b
                        own_kv_build(0, t_, hts, hcols=b * 128)
                        if t_ + 1 in qb[1:4]:
                            h_ = qb.index(t_ + 1) - 1
                            fire_ag(0, h_)
                            copy_ag(0, h_)

            HT = (tpc + 1) // 2  # tiles per self-batch half
            SELF_T = [(0, HT), (HT, tpc - HT)]

            def self_msgs(ell, hf):
                """Batched self-loop messages for tiles [t0, t0+nb).

                Returns (tile, stride, off): per-tile message is
                tile[:, (t-t0)*stride + off : ... + HID] (bf16, already
                premultiplied by dinv[own])."""
                t0, nb = SELF_T[hf]
                if ell == 1:
                    sv1 = sb_self2.tile([128, HT * HID], BF16, name="sv1")
                    nc.sync.dma_start(
                        sv1[:, :nb * HID].rearrange("p (b d) -> p b d", b=nb),
                        kvown[:].rearrange("(b p) e -> p b e", p=128)
                        [:, t0:t0 + nb, HID:2 * HID])
                    return sv1, HID, 0
                skv = sb_self.tile([128, HT * LMAX * 128], BF16, name="skv")
                nc.sync.dma_start(
                    skv[:, :nb * ell * 128].rearrange("p (b e) -> p b e", b=nb),
                    kvown[:].rearrange("(b p) e -> p b e", p=128)
                    [:, t0:t0 + nb, :ell * 128])
                kvvS = skv[:, :nb * ell * 128].rearrange(
                    "p (b l s h d) -> p b l s h d", b=nb, l=ell, s=2, h=H, d=DH)
                qbv = qstore[ell % 2][:, t0 * HID:(t0 + nb) * HID].rearrange(
                    "p (b u h d) -> p b u h d", b=nb, u=1, h=H).to_broadcast(
                    [128, nb, ell, H, DH])
                qks = sb_self2.tile([128, HT * LMAX * HID], BF16, name="qks")
                qksv = qks[:, :nb * ell * HID].rearrange(
                    "p (b l h d) -> p b l h d", b=nb, l=ell, h=H, d=DH)
                nc.vector.tensor_tensor(qksv, qbv, kvvS[:, :, :, 0, :, :], ALU.mult)
                ssS = sb_sm.tile([128, HT * LMAX * H], F32, name="ssS")
                nc.vector.reduce_sum(
                    ssS[:, :nb * ell * H].rearrange("p (b l h) -> p b l h",
                                                    b=nb, l=ell, h=H),
                    qksv, axis=AX.X)
                eS = sb_sm.tile([128, HT * LMAX * H], BF16, name="eS")
                nc.scalar.activation(eS[:, :nb * ell * H], ssS[:, :nb * ell * H],
                                     AF.Exp, scale=1.0 / math.sqrt(DH))
                denS = sb_sm.tile([128, HT * H], F32, name="denS")
                nc.vector.reduce_sum(
                    denS[:, :nb * H].rearrange("p (b h) -> p b h", b=nb),
                    eS[:, :nb * ell * H].rearrange("p (b l h) -> p b h l",
                                                   b=nb, l=ell, h=H),
                    axis=AX.X)
                rdenS = sb_sm.tile([128, HT * H], BF16, name="rdenS")
                with nc.allow_low_precision(reason="bf16 softmax weights"):
                    nc.vector.reciprocal(rdenS[:, :nb * H], denS[:, :nb * H])
                attS = sb_sm.tile([128, HT * LMAX * H], BF16, name="attS")
                atv = attS[:, :nb * ell * H].rearrange("p (b l h) -> p b l h",
                                                       b=nb, l=ell, h=H)
                nc.vector.tensor_tensor(
                    atv,
                    eS[:, :nb * ell * H].rearrange("p (b l h) -> p b l h",
                                                   b=nb, l=ell, h=H),
                    rdenS[:, :nb * H].rearrange("p (b u h) -> p b u h", b=nb, u=1)
                    .to_broadcast([128, nb, ell, H]),
                    ALU.mult)
                msgS = sb_self2.tile([128, HT * HID], BF16, name="msgS")
                wvS = sb_self2.tile([128, HT * HID], BF16, name="wvS")
                mS = msgS[:, :nb * HID].rearrange("p (b h d) -> p b h d", b=nb, h=H)
                wS = wvS[:, :nb * HID].rearrange("p (b h d) -> p b h d", b=nb, h=H)
                for l in range(ell):
                    nc.vector.tensor_tensor(
                        mS if l == 0 else wS,
                        atv[:, :, l, :].to_broadcast([128, nb, H, DH]),
                        kvvS[:, :, l, 1, :, :], ALU.mult)
                    if l > 0:
                        nc.vector.tensor_add(msgS[:, :nb * HID],
                                             msgS[:, :nb * HID],
                                             wvS[:, :nb * HID])
                return msgS, HID, 0

            def attn_block(ell, t, kvg, oo, cc, oh):
                L = ell
                qtile = qstore[ell % 2][:, t * HID:(t + 1) * HID]
                qe = ps_qe.tile([128, C_MAX * HID], F32, name="qe",
                                space="PSUM")
                for k in range(cc):
                    ohtp = ps_oht.tile([128, 128], BF16, name="ohtp",
                                       space="PSUM")
                    nc.tensor.transpose(ohtp[:],
                                        in_=oh[:, k * 128:(k + 1) * 128],
                                        identity=iden_bf[:])
                    ohts = sb_oh.tile([128, 128], BF16, name="ohts")
                    nc.scalar.copy(ohts[:], ohtp[:])
                    nc.tensor.matmul(qe[:, k * HID:(k + 1) * HID],
                                     lhsT=ohts[:], rhs=qtile,
                                     start=True, stop=True)

                qeb = sb_ve.tile([128, C_MAX * HID], BF16, name="qeb")
                nc.scalar.copy(qeb[:, :cc * HID], qe[:, :cc * HID])
                kvv = kvg[:, oo * L * 128:(oo + cc) * L * 128].rearrange(
                    "p (c l s h d) -> p c l s h d", c=cc, l=L, s=2, h=H, d=DH)
                qv = qeb[:, :cc * HID].rearrange("p (c h d) -> p c h d",
                                                 h=H, d=DH)
                satt = sb_sm.tile([128, C_MAX * H * LMAX], F32, name="satt")
                sv = satt[:, :cc * H * L].rearrange("p (c h l) -> p c h l",
                                                    h=H, l=L)
                qk = sb_ve.tile([128, C_MAX * HID], BF16, name="qk")
                qkv = qk[:, :cc * HID].rearrange("p (c h d) -> p c h d",
                                                 h=H, d=DH)
                for l in range(L):
                    nc.vector.tensor_tensor(qkv, qv, kvv[:, :, l, 0, :, :],
                                            ALU.mult)
                    nc.vector.reduce_sum(sv[:, :, :, l], qkv, axis=AX.X)
                eatt = sb_sm.tile([128, C_MAX * H * LMAX], BF16, name="eatt")
                nc.scalar.activation(eatt[:, :cc * H * L], satt[:, :cc * H * L],
                                     AF.Exp, scale=1.0 / math.sqrt(DH))
                den = sb_sm.tile([128, C_MAX * H], F32, name="den")
                nc.vector.reduce_sum(
                    den[:, :cc * H].rearrange("p (c h) -> p c h", h=H),
                    eatt[:, :cc * H * L].rearrange("p (c h l) -> p c h l",
                                                   h=H, l=L),
                    axis=AX.X)
                rden = sb_sm.tile([128, C_MAX * H], BF16, name="rden")
                with nc.allow_low_precision(
                        reason="bf16 softmax weights; rel-err gate 2e-2"):
                    nc.vector.reciprocal(rden[:, :cc * H], den[:, :cc * H])
                att = sb_sm.tile([128, C_MAX * H * LMAX], BF16, name="att")
                av = att[:, :cc * H * L].rearrange("p (c h l) -> p c h l",
                                                   h=H, l=L)
                nc.vector.tensor_tensor(
                    av,
                    eatt[:, :cc * H * L].rearrange("p (c h l) -> p c h l",
                                                   h=H, l=L),
                    rden[:, :cc * H].rearrange("p (c h u) -> p c h u",
                                               h=H, u=1)
                    .to_broadcast([128, cc, H, L]),
                    ALU.mult)
                msg = sb_ve.tile([128, C_MAX * HID], BF16, name="msg")
                wvt = sb_ve.tile([128, C_MAX * HID], BF16, name="wvt")
                mv = msg[:, :cc * HID].rearrange("p (c h d) -> p c h d",
                                                 h=H, d=DH)
                wvv = wvt[:, :cc * HID].rearrange("p (c h d) -> p c h d",
                                                  h=H, d=DH)
                for l in range(L):
                    nc.vector.tensor_tensor(
                        mv if l == 0 else wvv,
                        av[:, :, :, l].to_broadcast([128, cc, H, DH]),
                        kvv[:, :, l, 1, :, :], ALU.mult)
                    if l > 0:
                        nc.vector.tensor_add(msg[:, :cc * HID],
                                             msg[:, :cc * HID],
                                             wvt[:, :cc * HID])
                return lambda k: msg[:, k * HID:(k + 1) * HID]

            # ================= edge pass =================
            def edge_layer(ell):
                L = ell
                selfb = [self_msgs(ell, 0), self_msgs(ell, 1)]
                for t in range(tpc):
                    stile, sstride, soff = selfb[0 if t < HT else 1]
                    st0 = 0 if t < HT else HT
                    po = ps_out.tile([128, HID], F32, name="po", space="PSUM")
                    n_ch = kt[t]
                    base = chunk_base[t]
                    done = 0
                    wins = []
                    for seg_o, seg_n in ((0, kt_lo[t]), (kt_lo[t], kt_hi[t])):
                        o = 0
                        while o < seg_n:
                            w = min(GW, seg_n - o)
                            wins.append((seg_o + o, w))
                            o += w
                    for (wo, gw) in wins:
                        rb = 0 if wo < kt_lo[t] else half
                        wcb = base + wo
                        kvi = sb_idx.tile([128, GW * 8], I16, name="kvi")
                        nc.sync.dma_start(kvi[:, :gw * 8],
                                          kvidx_d[:, wcb * 8:(wcb + gw) * 8])
                        if ell == 1:
                            gt = sb_g.tile([128, GW * LMAX * 128], BF16, name="kvg")
                            nc.gpsimd.dma_gather(
                                out_ap=gt[:, :gw * 128].rearrange(
                                    "p (n d) -> p n d", d=128),
                                in_ap=kvtab[rb:, 0:128],
                                idxs_ap=kvi[:, :gw * 8],
                                num_idxs=gw * 128, num_idxs_reg=gw * 128,
                                elem_size=128, elem_step=KVROW,
                                queue_num=next_q())
                        else:
                            gt = sb_g.tile([128, GW * LMAX * 128], BF16, name="kvg")
                            nc.gpsimd.dma_gather(
                                out_ap=gt[:, :gw * L * 128].rearrange(
                                    "p (n d) -> p n d", d=L * 128),
                                in_ap=kvtab[rb:, :L * 128],
                                idxs_ap=kvi[:, :gw * 8],
                                num_idxs=gw * 128, num_idxs_reg=gw * 128,
                                elem_size=L * 128, elem_step=KVROW,
                                queue_num=next_q())
                        oo = 0
                        while oo < gw:
                            cc = min(C_MAX, gw - oo)
                            cb = wcb + oo
                            dli = sb_idx.tile([128, C_MAX], BF16, name="dli")
                            nc.sync.dma_start(dli[:, :cc], dlocT_d[:, cb:cb + cc])
                            oh = sb_oh.tile([128, C_MAX * 128], BF16, name="oh")
                            nc.vector.tensor_tensor(
                                oh[:, :cc * 128].rearrange("p (c i) -> p c i",
                                                           c=cc, i=128),
                                dli[:, :cc].rearrange("p (c u) -> p c u", c=cc, u=1)
                                .to_broadcast([128, cc, 128]),
                                iota_s[:].rearrange("p (u i) -> p u i", u=1, i=128)
                                .to_broadcast([128, cc, 128]),
                                ALU.is_equal)
                            if ell == 1:
                                vgv = gt[:, oo * 128:(oo + cc) * 128].rearrange(
                                    "p (c s d) -> p c s d", s=2, d=HID)
                                mslice = lambda k, vgv=vgv: vgv[:, k, 1, :]
                            else:
                                mslice = attn_block(ell, t, gt, oo, cc, oh)
                            for k in range(cc):
                                nc.tensor.matmul(po[:],
                                                 lhsT=oh[:, k * 128:(k + 1) * 128],
                                                 rhs=mslice(k),
                                                 start=(done + k == 0),
                                                 stop=(done + k == n_ch - 1))
                            done += cc
                            oo += cc

                    # out = (po + self) * dinv[dst] + corr[dst] * bv  (2 fused ops)
                    so = (t - st0) * sstride + soff
                    ob0 = sb_out.tile([128, HID], F32, name="ob0")
                    nc.vector.scalar_tensor_tensor(
                        out=ob0[:], in0=stile[:, so:so + HID],
                        scalar=dinvL_s[:, t:t + 1],
                        in1=biasT_s[:, t * HID:(t + 1) * HID],
                        op0=ALU.mult, op1=ALU.add)
                    outsb = sb_out.tile([128, HID], F32, name="outsb")
                    nc.vector.scalar_tensor_tensor(
                        out=outsb[:], in0=po[:], scalar=dinvL_s[:, t:t + 1],
                        in1=ob0[:], op0=ALU.mult, op1=ALU.add)
                    if ell < 3:
                        # shared transpose for q-build and kv-build
                        htp = ps_big.tile([HID, 512], F32, name="htp", space="PSUM")
                        nc.tensor.transpose(htp[:, :128], in_=outsb[:], identity=iden[:])
                        hts = sb_ht.tile([HID, 512], F32, name="hts")
                        nc.scalar.copy(hts[:, :128], htp[:, :128])
                        own_q_build((ell + 1) % 2, t, hts)
                        own_kv_build(ell, t, hts)
                        if t + 1 in qb[1:]:
                            h_ = qb.index(t + 1) - 1
                            fire_ag(ell, h_)
                            copy_ag(ell, h_)
                    else:
                        final_tile(outsb, t)

            def final_tile(outsb, t):
                htp = ps_big.tile([HID, 512], F32, name="htp", space="PSUM")
                nc.tensor.transpose(htp[:, :128], in_=outsb[:], identity=iden[:])
                hts = sb_ht.tile([HID, 512], F32, name="hts")
                nc.scalar.copy(hts[:, :128], htp[:, :128])
                yp = ps_kv.tile([128, 2 * HID], F32, name="kvp", space="PSUM")
                nc.tensor.matmul(yp[:, :OUTC], lhsT=hts[:, :128], rhs=wout_s[:],
                                 start=True, stop=False)
                nc.tensor.matmul(yp[:, :OUTC], lhsT=ones_r[:], rhs=bout_s[:],
                                 start=False, stop=True)
                ysb = sb_out.tile([128, OUTC], F32, name="ysb")
                nc.scalar.copy(ysb[:], yp[:, :OUTC])
                nc.sync.dma_start(y[t * 128:(t + 1) * 128, :], ysb[:])

            # ================= schedule =================
            build_slice0()
            fire_ag(0, 3)
            copy_ag(0, 3)
            edge_layer(1)
            edge_layer(2)
            edge_layer(3)

    nc.compile()
    return nc


def assemble_output(cfg, results):
    n = cfg["n_nodes"]
    out = np.zeros((n, OUTC), np.float32)
    for c in range(cfg["n_cores"]):
        cn = cfg["core_nodes"][c]
        out[cn] = results[c]["y"][:len(cn)]
    return out


# ======================= harness entry point =======================
LAST_EXEC_NS = [None]
LAST_RESULT = [None]


def kernel(**inputs):
    """Full (unsharded) inputs -> full [N, 16] float32 output."""
    from concourse.bass_utils import run_bass_kernel_spmd

    x = np.asarray(inputs["x"], np.float32)
    edge_index = np.asarray(inputs["edge_index"])
    cfg = make_cfg(x.shape[0], edge_index, n_cores=8)
    in_maps = prep_inputs(
        cfg, x,
        inputs["W_lin"], inputs["b_lin"],
        inputs["Wq"], inputs["bq"],
        inputs["Wk"], inputs["bk"],
        inputs["Wv"], inputs["bv"],
        inputs["W_out"], inputs["b_out"],
    )
    nc = build_kernel(cfg)
    res = run_bass_kernel_spmd(nc, in_maps, core_ids=list(range(cfg["n_cores"])))
    LAST_EXEC_NS[0] = res.exec_time_ns
    LAST_RESULT[0] = res
    return assemble_output(cfg, res.results)


# revision 5
# speedup vs baseline: 1.1647x; 1.0015x over previous
"""DNANet Bass kernel v3.

Changes vs v2:
- Node->slot remap: nodes degree-sorted and snake-dealt across cores so
  per-core edge counts (and per-tile chunk counts) match across cores;
  kvtab rows are in slot order.
- Each core builds k/v only for its OWN nodes (interleaved into the edge
  pass, sharing the q-build transpose), then the k/v table slice is
  AllGathered (in two halves, first half overlapped with the edge pass)
  and copied into the interleaved kvtab columns.  This removes the
  ~0.9ms all-nodes table rebuild between layers.
- bk dropped entirely (softmax over layers is invariant to the q.bk
  shift); bv folded into a host-precomputed rank-1 per-node correction
  (sum of attention weights is 1), applied with one fused
  scalar_tensor_tensor per tile.
- q tiles stay resident in SBUF (no DRAM round trip).
- Gather pad indices are negative (skipped by SWDGE) to trim Q7
  descriptor-generation time.
- onehot inputs in bf16 for 2x DVE rate; PSUM->SBUF casts moved to the
  (otherwise idle) scalar engine.
"""
import math
import numpy as np
import concourse.bacc as bacc
import concourse.mybir as mybir
import concourse.tile as tile
from concourse.masks import make_identity

F32, I32, I16 = mybir.dt.float32, mybir.dt.int32, mybir.dt.int16
BF16 = mybir.dt.bfloat16
HID, H, DH, INC, OUTC = 64, 4, 16, 128, 16
LMAX = 3
KVROW = LMAX * 2 * HID  # 384 elements per node row in kv table
HALF = 25088            # row split so int16 indices stay positive
C_MAX = 8               # chunks per compute superchunk
GW = 8                  # chunks per gather window (<=1024 idx per dma_gather)


def _wrap16(arr_cm):
    """[tot_ch, 128] int -> [128, tot_ch*8] int16 ucode idx layout."""
    tc_, _ = arr_cm.shape
    a = arr_cm.reshape(tc_, 8, 16).transpose(2, 0, 1).reshape(16, tc_ * 8)
    out = np.zeros((128, tc_ * 8), np.int16)
    for r in range(8):
        out[r * 16:(r + 1) * 16] = a
    return out


def make_cfg(n_nodes, edge_index, n_cores=8):
    tpc = math.ceil(n_nodes / n_cores / 128)
    npad = n_cores * tpc * 128
    nsh = tpc * 128
    half = HALF if npad + 128 > 32000 else npad + 128

    src_e = np.asarray(edge_index[0], dtype=np.int64)
    dst_e = np.asarray(edge_index[1], dtype=np.int64)
    loop = np.arange(n_nodes, dtype=np.int64)
    src = np.concatenate([src_e, loop])
    dst = np.concatenate([dst_e, loop])

    deg = np.bincount(dst, minlength=n_nodes).astype(np.float32)
    dinv = np.where(deg > 0, 1.0 / np.sqrt(np.maximum(deg, 1.0)), 0.0).astype(np.float32)

    # ---- node -> (core, slot) snake deal by degree --------------------
    order = np.argsort(-deg, kind="stable")  # high degree first
    slot_of = np.full(npad, -1, np.int64)
    core_nodes = [[] for _ in range(n_cores)]
    for r in range(0, n_nodes, n_cores):
        blk = order[r:r + n_cores]
        cs = range(n_cores) if (r // n_cores) % 2 == 0 else range(n_cores - 1, -1, -1)
        for c, nd in zip(cs, blk):
            core_nodes[c].append(nd)
    for c in range(n_cores):
        cn = np.asarray(core_nodes[c], np.int64)
        slot_of[cn] = c * nsh + np.arange(len(cn))
        core_nodes[c] = cn
    # dinv in slot order (pad slots -> 0)
    dinv_slot = np.zeros(npad, np.float32)
    for c in range(n_cores):
        dinv_slot[c * nsh:c * nsh + len(core_nodes[c])] = dinv[core_nodes[c]]

    # chunk path uses only the original edges; the added self-loops are
    # handled by the batched self path (own k/v rows, no gather)
    src_sl = slot_of[src_e]
    dst_sl = slot_of[dst_e]

    # bias correction: corr[n] = dinv[n] * sum_{e->n} dinv[src]
    ssum = np.zeros(n_nodes, np.float64)
    np.add.at(ssum, dst, dinv[src].astype(np.float64))
    corr = dinv * ssum.astype(np.float32)
    corr_slot = np.zeros(npad, np.float32)
    for c in range(n_cores):
        corr_slot[c * nsh:c * nsh + len(core_nodes[c])] = corr[core_nodes[c]]

    key = dst_sl * 2 + (src_sl >= half)
    order_e = np.argsort(key, kind="stable")
    src_s, dst_s = src_sl[order_e], dst_sl[order_e]

    cnt_lo = np.zeros((n_cores, tpc), np.int64)
    cnt_hi = np.zeros((n_cores, tpc), np.int64)
    core_of = dst_s // nsh
    ltile = (dst_s % nsh) // 128
    is_hi = src_s >= half
    np.add.at(cnt_lo, (core_of[~is_hi], ltile[~is_hi]), 1)
    np.add.at(cnt_hi, (core_of[is_hi], ltile[is_hi]), 1)
    kt_lo = np.maximum(np.ceil(cnt_lo.max(axis=0) / 128).astype(np.int64), 1)
    kt_hi = np.ceil(cnt_hi.max(axis=0) / 128).astype(np.int64)
    kt = kt_lo + kt_hi
    tot_ch = int(kt.sum())
    chunk_base = np.concatenate([[0], np.cumsum(kt)]).astype(np.int64)

    kvidx_cm = np.zeros((n_cores, tot_ch, 128), np.int64)  # pad -> row 0 (masked)
    dloc_cm = np.full((n_cores, tot_ch, 128), 128, np.int64)
    for c in range(n_cores):
        lo = np.searchsorted(dst_s, c * nsh)
        hi_ = np.searchsorted(dst_s, (c + 1) * nsh)
        sseg_c = src_s[lo:hi_]
        dseg_c = dst_s[lo:hi_]
        for t in range(tpc):
            g0 = c * nsh + t * 128
            l0 = np.searchsorted(dseg_c, g0)
            l1 = np.searchsorted(dseg_c, g0 + 128)
            if l1 == l0:
                continue
            sseg = sseg_c[l0:l1]
            dseg = dseg_c[l0:l1]
            hseg = sseg >= half
            for half_i, mask, base_ch in (
                (0, ~hseg, chunk_base[t]),
                (1, hseg, chunk_base[t] + kt_lo[t]),
            ):
                sv = sseg[mask]
                dv = dseg[mask]
                n = len(sv)
                if n == 0:
                    continue
                ch = base_ch + np.arange(n) // 128
                lane = np.arange(n) % 128
                kvidx_cm[c, ch, lane] = sv - (half if half_i else 0)
                dloc_cm[c, ch, lane] = dv - g0

    # gather windows (same enumeration as edge_layer) + per-core valid counts
    scs_all = []  # (tile, chunk_base+offset, cc) in issue order
    for t in range(tpc):
        for seg_o, seg_n in ((0, kt_lo[t]), (kt_lo[t], kt_hi[t])):
            o = 0
            while o < seg_n:
                w = min(GW, seg_n - o)
                scs_all.append((t, int(chunk_base[t]) + seg_o + o, w))
                o += w
    n_g = len(scs_all)

    return dict(
        n_cores=n_cores, tpc=tpc, npad=npad, nsh=nsh, n_nodes=n_nodes,
        half=half,
        kt=[int(k) for k in kt], kt_lo=[int(k) for k in kt_lo],
        kt_hi=[int(k) for k in kt_hi],
        tot_ch=tot_ch, chunk_base=[int(b) for b in chunk_base],
        n_g=n_g,
        kvidx=np.stack([_wrap16(kvidx_cm[c].astype(np.int16)) for c in range(n_cores)]),
        dlocT=np.ascontiguousarray(
            dloc_cm.astype(np.int32).transpose(0, 2, 1)),  # [c, 128, tot_ch]
        dinv_slot=dinv_slot, corr_slot=corr_slot,
        core_nodes=core_nodes,
    )


def prep_inputs(cfg, x, W_lin, b_lin, Wq, bq, Wk, bk, Wv, bv, W_out, b_out):
    ncore, npad, tpc, nsh = cfg["n_cores"], cfg["npad"], cfg["tpc"], cfg["nsh"]
    x = np.asarray(x, np.float32)
    bv = np.asarray(bv, np.float32)
    import ml_dtypes
    iota = np.broadcast_to(np.arange(128, dtype=np.float32), (128, 128))
    base = {
        "wlin": np.asarray(W_lin, np.float32),
        "wq": np.asarray(Wq, np.float32),
        "wk": np.asarray(Wk, np.float32),
        "wv": np.asarray(Wv, np.float32),
        "wout": np.asarray(W_out, np.float32),
        "blin_col": np.asarray(b_lin, np.float32).reshape(HID, 1),
        "bq_r": np.asarray(bq, np.float32).reshape(1, HID),
        "bout_r": np.asarray(b_out, np.float32).reshape(1, OUTC),
        "iota_bf": iota.astype(ml_dtypes.bfloat16),  # values 0..127 exact in bf16
    }
    in_maps = []
    for c in range(ncore):
        m = dict(base)
        cn = cfg["core_nodes"][c]
        xTL = np.zeros((INC, nsh), np.float32)
        xTL[:, :len(cn)] = x[cn].T
        m["xTL"] = xTL
        dsl = cfg["dinv_slot"][c * nsh:(c + 1) * nsh]
        m["dinvL"] = np.ascontiguousarray(dsl.reshape(tpc, 128).T)  # [128, tpc]
        csl = cfg["corr_slot"][c * nsh:(c + 1) * nsh]
        # biasT[p, t*64+j] = corr[slot t*128+p] * bv[j]
        bt = csl.reshape(tpc, 128)[:, :, None] * bv[None, None, :]  # [tpc,128,64]
        m["biasT"] = np.ascontiguousarray(
            bt.transpose(1, 0, 2).reshape(128, tpc * HID)).astype(np.float32)
        m["kvidx"] = cfg["kvidx"][c]
        # dloc as bf16 (values 0..128 exact)
        m["dlocT"] = cfg["dlocT"][c].astype(ml_dtypes.bfloat16)
        in_maps.append(m)
    return in_maps


def build_kernel(cfg):
    ncore, tpc, npad, nsh = cfg["n_cores"], cfg["tpc"], cfg["npad"], cfg["nsh"]
    kt, kt_lo, kt_hi = cfg["kt"], cfg["kt_lo"], cfg["kt_hi"]
    tot_ch, chunk_base, half = cfg["tot_ch"], cfg["chunk_base"], cfg["half"]

    # AllGather quarters: tile group boundaries
    qb = [0, 2 * tpc // 7, 4 * tpc // 7, 6 * tpc // 7, tpc]
    rows_h = [(qb[i + 1] - qb[i]) * 128 for i in range(4)]
    NQ = 4

    nc = bacc.Bacc("TRN2", target_bir_lowering=False, debug=False,
                   num_devices=ncore, num_swdge_queues=4)

    xTL = nc.dram_tensor("xTL", [INC, nsh], F32, kind="ExternalInput")
    wlin = nc.dram_tensor("wlin", [INC, HID], F32, kind="ExternalInput")
    wq = nc.dram_tensor("wq", [HID, HID], F32, kind="ExternalInput")
    wk = nc.dram_tensor("wk", [HID, HID], F32, kind="ExternalInput")
    wv = nc.dram_tensor("wv", [HID, HID], F32, kind="ExternalInput")
    wout = nc.dram_tensor("wout", [HID, OUTC], F32, kind="ExternalInput")
    blin_col = nc.dram_tensor("blin_col", [HID, 1], F32, kind="ExternalInput")
    bq_r = nc.dram_tensor("bq_r", [1, HID], F32, kind="ExternalInput")
    bout_r = nc.dram_tensor("bout_r", [1, OUTC], F32, kind="ExternalInput")
    iota_bf = nc.dram_tensor("iota_bf", [128, 128], BF16, kind="ExternalInput")
    dinvL_d = nc.dram_tensor("dinvL", [128, tpc], F32, kind="ExternalInput")
    biasT_d = nc.dram_tensor("biasT", [128, tpc * HID], F32, kind="ExternalInput")
    kvidx_d = nc.dram_tensor("kvidx", [128, tot_ch * 8], I16, kind="ExternalInput")
    dlocT_d = nc.dram_tensor("dlocT", [128, tot_ch], BF16, kind="ExternalInput")
    y = nc.dram_tensor("y", [nsh, OUTC], F32, kind="ExternalOutput")

    with tile.TileContext(nc) as tc:
        import contextlib
        ctx = contextlib.ExitStack()
        with ctx:
            cpool = ctx.enter_context(tc.tile_pool(name="const", bufs=1))
            dram = ctx.enter_context(tc.tile_pool(name="dram", bufs=1, space="DRAM"))

            kvtab = dram.tile([npad + 128, KVROW], BF16, name="kvtab")
            kvown = dram.tile([nsh, KVROW], BF16, name="kvown")
            # per (slice, quarter) collective buffers
            agin = [[dram.tile([rows_h[h], 128], BF16, name=f"agin{s}_{h}")
                     for h in range(NQ)] for s in (0, 1, 2)]
            agout = [[dram.tile([ncore * rows_h[h], 128], BF16,
                                name=f"agout{s}_{h}", addr_space="Shared")
                      for h in range(NQ)] for s in (0, 1, 2)]

            def load_const(dt_, shape, src_ap, name):
                t_ = cpool.tile(shape, dt_, name=name)
                nc.sync.dma_start(t_[:], src_ap)
                return t_

            wlin_s = load_const(F32, [INC, HID], wlin[:], "wlin_s")
            wq_s = load_const(F32, [HID, HID], wq[:], "wq_s")
            wk_s = load_const(F32, [HID, HID], wk[:], "wk_s")
            wv_s = load_const(F32, [HID, HID], wv[:], "wv_s")
            wout_s = load_const(F32, [HID, OUTC], wout[:], "wout_s")
            blin_s = load_const(F32, [HID, 1], blin_col[:], "blin_s")
            bq_s = load_const(F32, [1, HID], bq_r[:], "bq_s")
            bout_s = load_const(F32, [1, OUTC], bout_r[:], "bout_s")
            iota_s = load_const(BF16, [128, 128], iota_bf[:], "iota_s")
            dinvL_s = load_const(F32, [128, tpc], dinvL_d[:], "dinvL_s")
            biasT_s = load_const(F32, [128, tpc * HID], biasT_d[:], "biasT_s")
            iden = cpool.tile([128, 128], F32, name="iden")
            make_identity(nc, iden[:])
            iden_bf = cpool.tile([128, 128], BF16, name="iden_bf")
            nc.vector.tensor_copy(iden_bf[:], iden[:])
            ones_r = cpool.tile([1, 128], F32, name="ones_r")
            nc.vector.memset(ones_r[:], 1.0)
            # SBUF-resident q tiles for layers 2 and 3
            qstore = [cpool.tile([128, tpc * HID], BF16, name=f"qstore{i}")
                      for i in (0, 1)]

            sb_xt = ctx.enter_context(tc.tile_pool(name="sb_xt", bufs=3))
            sb_ht = ctx.enter_context(tc.tile_pool(name="sb_ht", bufs=3))
            sb_kv = ctx.enter_context(tc.tile_pool(name="sb_kv", bufs=3))
            sb_idx = ctx.enter_context(tc.tile_pool(name="sb_idx", bufs=8))
            sb_oh = ctx.enter_context(tc.tile_pool(name="sb_oh", bufs=6))
            sb_g = ctx.enter_context(tc.tile_pool(name="sb_g", bufs=5))
            sb_ve = ctx.enter_context(tc.tile_pool(name="sb_ve", bufs=5))
            sb_sm = ctx.enter_context(tc.tile_pool(name="sb_sm", bufs=6))
            sb_out = ctx.enter_context(tc.tile_pool(name="sb_out", bufs=3))
            sb_self = ctx.enter_context(tc.tile_pool(name="sb_self", bufs=2))
            sb_self2 = ctx.enter_context(tc.tile_pool(name="sb_self2", bufs=2))
            ps_big = ctx.enter_context(tc.tile_pool(name="ps_big", bufs=1, space="PSUM"))
            ps_kv = ctx.enter_context(tc.tile_pool(name="ps_kv", bufs=2, space="PSUM"))
            ps_out = ctx.enter_context(tc.tile_pool(name="ps_out", bufs=1, space="PSUM"))
            ps_qe = ctx.enter_context(tc.tile_pool(name="ps_qe", bufs=2, space="PSUM"))
            ps_oht = ctx.enter_context(tc.tile_pool(name="ps_oht", bufs=2, space="PSUM"))

            AF, ALU = mybir.ActivationFunctionType, mybir.AluOpType
            AX = mybir.AxisListType
            qctr = [0]

            def next_q():
                qctr[0] += 1
                return qctr[0] % 4

            def agslice(s, t):
                """(quarter, row0) for tile t in the slice-s AG input."""
                for h in range(NQ):
                    if t < qb[h + 1]:
                        return h, (t - qb[h]) * 128
                raise AssertionError(t)

            # ---- per-own-tile k/v (+q) build from hts [HID, 128] -------
            def own_kv_build(s, t, hts, hcols=0):
                kvp = ps_kv.tile([128, 2 * HID], F32, name="kvp", space="PSUM")
                nc.tensor.matmul(kvp[:, :HID], lhsT=hts[:, hcols:hcols + 128],
                                 rhs=wk_s[:], start=True, stop=True)
                nc.tensor.matmul(kvp[:, HID:], lhsT=hts[:, hcols:hcols + 128],
                                 rhs=wv_s[:], start=True, stop=True)
                kvsb = sb_kv.tile([128, 128], BF16, name="kvsb")
                nc.scalar.copy(kvsb[:, :HID], kvp[:, :HID])
                nc.scalar.mul(kvsb[:, HID:], kvp[:, HID:],
                              dinvL_s[:, t:t + 1])
                h_, r0 = agslice(s, t)
                nc.sync.dma_start(agin[s][h_][r0:r0 + 128, :], kvsb[:])
                nc.sync.dma_start(
                    kvown[t * 128:(t + 1) * 128, s * 128:(s + 1) * 128], kvsb[:])

            def own_q_build(qi, t, hts):
                qp = ps_kv.tile([128, 2 * HID], F32, name="kvp", space="PSUM")
                nc.tensor.matmul(qp[:, :HID], lhsT=hts[:, :128], rhs=wq_s[:],
                                 start=True, stop=False)
                nc.tensor.matmul(qp[:, :HID], lhsT=ones_r[:], rhs=bq_s[:],
                                 start=False, stop=True)
                nc.scalar.copy(qstore[qi][:, t * HID:(t + 1) * HID], qp[:, :HID])

            def fire_ag(s, h_):
                nc.gpsimd.collective_compute(
                    "AllGather", mybir.AluOpType.bypass,
                    replica_groups=[list(range(ncore))],
                    ins=[agin[s][h_].opt()], outs=[agout[s][h_].opt()])

            def copy_ag(s, h_):
                # copy into kvtab columns [s*128:(s+1)*128], rows by core block
                rh = rows_h[h_]
                off = qb[h_] * 128
                src = agout[s][h_][:].rearrange("(c r) d -> c r d", c=ncore)
                dst = kvtab[0:npad, s * 128:(s + 1) * 128].rearrange(
                    "(c r) d -> c r d", c=ncore, r=nsh)[:, off:off + rh, :]
                nc.sync.dma_start(dst, src)

            # ================= slice 0 (h = x @ W_lin + b) ==============
            def build_slice0():
                ng = nsh // 512  # 12 full groups of 512
                rem = (nsh - ng * 512) // 128
                for g in range(ng + 1):
                    nb = 4 if g < ng else rem
                    if nb == 0:
                        continue
                    w = nb * 128
                    xt_t = sb_xt.tile([INC, 512], F32, name="xt_t")
                    nc.sync.dma_start(xt_t[:, :w], xTL[:, g * 512:g * 512 + w])
                    htp = ps_big.tile([HID, 512], F32, name="htp", space="PSUM")
                    for b in range(nb):
                        nc.tensor.matmul(htp[:, b * 128:(b + 1) * 128], lhsT=wlin_s[:],
                                         rhs=xt_t[:, b * 128:(b + 1) * 128],
                                         start=True, stop=True)
                    hts = sb_ht.tile([HID, 512], F32, name="hts")
                    nc.vector.tensor_scalar(out=hts[:, :w], in0=htp[:, :w],
                                            scalar1=blin_s[:], scalar2=None,
                                            op0=ALU.add)
                    for b in range(nb):
                        t_ = g * 4 + b
                        own_kv_build(0, t_, hts, hcols=b * 128)
                        if t_ + 1 in qb[1:4]:
                            h_ = qb.index(t_ + 1) - 1
                            fire_ag(0, h_)
                            copy_ag(0, h_)

            HT = (tpc + 1) // 2  # tiles per self-batch half
            SELF_T = [(0, HT), (HT, tpc - HT)]

            def self_msgs(ell, hf):
                """Batched self-loop messages for tiles [t0, t0+nb).

                Returns (tile, stride, off): per-tile message is
                tile[:, (t-t0)*stride + off : ... + HID] (bf16, already
                premultiplied by dinv[own])."""
                t0, nb = SELF_T[hf]
                if ell == 1:
                    sv1 = sb_self2.tile([128, HT * HID], BF16, name="sv1")
                    nc.sync.dma_start(
                        sv1[:, :nb * HID].rearrange("p (b d) -> p b d", b=nb),
                        kvown[:].rearrange("(b p) e -> p b e", p=128)
                        [:, t0:t0 + nb, HID:2 * HID])
                    return sv1, HID, 0
                skv = sb_self.tile([128, HT * LMAX * 128], BF16, name="skv")
                nc.sync.dma_start(
                    skv[:, :nb * ell * 128].rearrange("p (b e) -> p b e", b=nb),
                    kvown[:].rearrange("(b p) e -> p b e", p=128)
                    [:, t0:t0 + nb, :ell * 128])
                kvvS = skv[:, :nb * ell * 128].rearrange(
                    "p (b l s h d) -> p b l s h d", b=nb, l=ell, s=2, h=H, d=DH)
                qbv = qstore[ell % 2][:, t0 * HID:(t0 + nb) * HID].rearrange(
                    "p (b u h d) -> p b u h d", b=nb, u=1, h=H).to_broadcast(
                    [128, nb, ell, H, DH])
                qks = sb_self2.tile([128, HT * LMAX * HID], BF16, name="qks")
                qksv = qks[:, :nb * ell * HID].rearrange(
                    "p (b l h d) -> p b l h d", b=nb, l=ell, h=H, d=DH)
                nc.vector.tensor_tensor(qksv, qbv, kvvS[:, :, :, 0, :, :], ALU.mult)
                ssS = sb_sm.tile([128, HT * LMAX * H], F32, name="ssS")
                nc.vector.reduce_sum(
                    ssS[:, :nb * ell * H].rearrange("p (b l h) -> p b l h",
                                                    b=nb, l=ell, h=H),
                    qksv, axis=AX.X)
                eS = sb_sm.tile([128, HT * LMAX * H], BF16, name="eS")
                nc.scalar.activation(eS[:, :nb * ell * H], ssS[:, :nb * ell * H],
                                     AF.Exp, scale=1.0 / math.sqrt(DH))
                denS = sb_sm.tile([128, HT * H], F32, name="denS")
                nc.vector.reduce_sum(
                    denS[:, :nb * H].rearrange("p (b h) -> p b h", b=nb),
                    eS[:, :nb * ell * H].rearrange("p (b l h) -> p b h l",
                                                   b=nb, l=ell, h=H),
                    axis=AX.X)
                rdenS = sb_sm.tile([128, HT * H], BF16, name="rdenS")
                with nc.allow_low_precision(reason="bf16 softmax weights"):
                    nc.vector.reciprocal(rdenS[:, :nb * H], denS[:, :nb * H])
                attS = sb_sm.tile([128, HT * LMAX * H], BF16, name="attS")
                atv = attS[:, :nb * ell * H].rearrange("p (b l h) -> p b l h",
                                                       b=nb, l=ell, h=H)
                nc.vector.tensor_tensor(
                    atv,
                    eS[:, :nb * ell * H].rearrange("p (b l h) -> p b l h",
                                                   b=nb, l=ell, h=H),
                    rdenS[:, :nb * H].rearrange("p (b u h) -> p b u h", b=nb, u=1)
                    .to_broadcast([128, nb, ell, H]),
                    ALU.mult)
                msgS = sb_self2.tile([128, HT * HID], BF16, name="msgS")
                wvS = sb_self2.tile([128, HT * HID], BF16, name="wvS")
                mS = msgS[:, :nb * HID].rearrange("p (b h d) -> p b h d", b=nb, h=H)
                wS = wvS[:, :nb * HID].rearrange("p (b h d) -> p b h d", b=nb, h=H)
                for l in range(ell):
                    nc.vector.tensor_tensor(
                        mS if l == 0 else wS,
                        atv[:, :, l, :].to_broadcast([128, nb, H, DH]),
                        kvvS[:, :, l, 1, :, :], ALU.mult)
                    if l > 0:
                        nc.vector.tensor_add(msgS[:, :nb * HID],
                                             msgS[:, :nb * HID],
                                             wvS[:, :nb * HID])
                return msgS, HID, 0

            def attn_block(ell, t, kvg, oo, cc, oh):
                L = ell
                qtile = qstore[ell % 2][:, t * HID:(t + 1) * HID]
                qe = ps_qe.tile([128, C_MAX * HID], F32, name="qe",
                                space="PSUM")
                for k in range(cc):
                    ohtp = ps_oht.tile([128, 128], BF16, name="ohtp",
                                       space="PSUM")
                    nc.tensor.transpose(ohtp[:],
                                        in_=oh[:, k * 128:(k + 1) * 128],
                                        identity=iden_bf[:])
                    ohts = sb_oh.tile([128, 128], BF16, name="ohts")
                    nc.scalar.copy(ohts[:], ohtp[:])
                    nc.tensor.matmul(qe[:, k * HID:(k + 1) * HID],
                                     lhsT=ohts[:], rhs=qtile,
                                     start=True, stop=True)

                qeb = sb_ve.tile([128, C_MAX * HID], BF16, name="qeb")
                nc.scalar.copy(qeb[:, :cc * HID], qe[:, :cc * HID])
                kvv = kvg[:, oo * L * 128:(oo + cc) * L * 128].rearrange(
                    "p (c l s h d) -> p c l s h d", c=cc, l=L, s=2, h=H, d=DH)
                qv = qeb[:, :cc * HID].rearrange("p (c h d) -> p c h d",
                                                 h=H, d=DH)
                satt = sb_sm.tile([128, C_MAX * H * LMAX], F32, name="satt")
                sv = satt[:, :cc * H * L].rearrange("p (c h l) -> p c h l",
                                                    h=H, l=L)
                qk = sb_ve.tile([128, C_MAX * HID], BF16, name="qk")
                qkv = qk[:, :cc * HID].rearrange("p (c h d) -> p c h d",
                                                 h=H, d=DH)
                for l in range(L):
                    nc.vector.tensor_tensor(qkv, qv, kvv[:, :, l, 0, :, :],
                                            ALU.mult)
                    nc.vector.reduce_sum(sv[:, :, :, l], qkv, axis=AX.X)
                eatt = sb_sm.tile([128, C_MAX * H * LMAX], BF16, name="eatt")
                nc.scalar.activation(eatt[:, :cc * H * L], satt[:, :cc * H * L],
                                     AF.Exp, scale=1.0 / math.sqrt(DH))
                den = sb_sm.tile([128, C_MAX * H], F32, name="den")
                nc.vector.reduce_sum(
                    den[:, :cc * H].rearrange("p (c h) -> p c h", h=H),
                    eatt[:, :cc * H * L].rearrange("p (c h l) -> p c h l",
                                                   h=H, l=L),
                    axis=AX.X)
                rden = sb_sm.tile([128, C_MAX * H], BF16, name="rden")
                with nc.allow_low_precision(
                        reason="bf16 softmax weights; rel-err gate 2e-2"):
                    nc.vector.reciprocal(rden[:, :cc * H], den[:, :cc * H])
                att = sb_sm.tile([128, C_MAX * H * LMAX], BF16, name="att")
                av = att[:, :cc * H * L].rearrange("p (c h l) -> p c h l",
                                                   h=H, l=L)
                nc.vector.tensor_tensor(
                    av,
                    eatt[:, :cc * H * L].rearrange("p (c h l) -> p c h l",
                                                   h=H, l=L),
                    rden[:, :cc * H].rearrange("p (c h u) -> p c h u",
                                               h=H, u=1)
                    .to_broadcast([128, cc, H, L]),
                    ALU.mult)
                msg = sb_ve.tile([128, C_MAX * HID], BF16, name="msg")
                wvt = sb_ve.tile([128, C_MAX * HID], BF16, name="wvt")
                mv = msg[:, :cc * HID].rearrange("p (c h d) -> p c h d",
                                                 h=H, d=DH)
                wvv = wvt[:, :cc * HID].rearrange("p (c h d) -> p c h d",
                                                  h=H, d=DH)
                for l in range(L):
                    nc.vector.tensor_tensor(
                        mv if l == 0 else wvv,
                        av[:, :, :, l].to_broadcast([128, cc, H, DH]),
                        kvv[:, :, l, 1, :, :], ALU.mult)
                    if l > 0:
                        nc.vector.tensor_add(msg[:, :cc * HID],
                                             msg[:, :cc * HID],
                                             wvt[:, :cc * HID])
                return lambda k: msg[:, k * HID:(k + 1) * HID]

            # ================= edge pass =================
            def edge_layer(ell):
                L = ell
                selfb = [self_msgs(ell, 0), self_msgs(ell, 1)]
                for t in range(tpc):
                    stile, sstride, soff = selfb[0 if t < HT else 1]
                    st0 = 0 if t < HT else HT
                    po = ps_out.tile([128, HID], F32, name="po", space="PSUM")
                    n_ch = kt[t]
                    base = chunk_base[t]
                    done = 0
                    wins = []
                    for seg_o, seg_n in ((0, kt_lo[t]), (kt_lo[t], kt_hi[t])):
                        o = 0
                        while o < seg_n:
                            w = min(GW, seg_n - o)
                            wins.append((seg_o + o, w))
                            o += w
                    for (wo, gw) in wins:
                        rb = 0 if wo < kt_lo[t] else half
                        wcb = base + wo
                        kvi = sb_idx.tile([128, GW * 8], I16, name="kvi")
                        nc.sync.dma_start(kvi[:, :gw * 8],
                                          kvidx_d[:, wcb * 8:(wcb + gw) * 8])
                        if ell == 1:
                            gt = sb_g.tile([128, GW * LMAX * 128], BF16, name="kvg")
                            nc.gpsimd.dma_gather(
                                out_ap=gt[:, :gw * 128].rearrange(
                                    "p (n d) -> p n d", d=128),
                                in_ap=kvtab[rb:, 0:128],
                                idxs_ap=kvi[:, :gw * 8],
                                num_idxs=gw * 128, num_idxs_reg=gw * 128,
                                elem_size=128, elem_step=KVROW,
                                queue_num=next_q())
                        else:
                            gt = sb_g.tile([128, GW * LMAX * 128], BF16, name="kvg")
                            nc.gpsimd.dma_gather(
                                out_ap=gt[:, :gw * L * 128].rearrange(
                                    "p (n d) -> p n d", d=L * 128),
                                in_ap=kvtab[rb:, :L * 128],
                                idxs_ap=kvi[:, :gw * 8],
                                num_idxs=gw * 128, num_idxs_reg=gw * 128,
                                elem_size=L * 128, elem_step=KVROW,
                                queue_num=next_q())
                        oo = 0
                        while oo < gw:
                            cc = min(C_MAX, gw - oo)
                            cb = wcb + oo
                            dli = sb_idx.tile([128, C_MAX], BF16, name="dli")
                            nc.sync.dma_start(dli[:, :cc], dlocT_d[:, cb:cb + cc])
                            oh = sb_oh.tile([128, C_MAX * 128], BF16, name="oh")
                            nc.vector.tensor_tensor(
                                oh[:, :cc * 128].rearrange("p (c i) -> p c i",
                                                           c=cc, i=128),
                                dli[:, :cc].rearrange("p (c u) -> p c u", c=cc, u=1)
                                .to_broadcast([128, cc, 128]),
                                iota_s[:].rearrange("p (u i) -> p u i", u=1, i=128)
                                .to_broadcast([128, cc, 128]),
                                ALU.is_equal)
                            if ell == 1:
                                vgv = gt[:, oo * 128:(oo + cc) * 128].rearrange(
                                    "p (c s d) -> p c s d", s=2, d=HID)
                                mslice = lambda k, vgv=vgv: vgv[:, k, 1, :]
                            else:
                                mslice = attn_block(ell, t, gt, oo, cc, oh)
                            for k in range(cc):
                                nc.tensor.matmul(po[:],
                                                 lhsT=oh[:, k * 128:(k + 1) * 128],
                                                 rhs=mslice(k),
                                                 start=(done + k == 0),
                                                 stop=(done + k == n_ch - 1))
                            done += cc
                            oo += cc

                    # out = (po + self) * dinv[dst] + corr[dst] * bv  (2 fused ops)
                    so = (t - st0) * sstride + soff
                    ob0 = sb_out.tile([128, HID], F32, name="ob0")
                    nc.vector.scalar_tensor_tensor(
                        out=ob0[:], in0=stile[:, so:so + HID],
                        scalar=dinvL_s[:, t:t + 1],
                        in1=biasT_s[:, t * HID:(t + 1) * HID],
                        op0=ALU.mult, op1=ALU.add)
                    outsb = sb_out.tile([128, HID], F32, name="outsb")
                    nc.vector.scalar_tensor_tensor(
                        out=outsb[:], in0=po[:], scalar=dinvL_s[:, t:t + 1],
                        in1=ob0[:], op0=ALU.mult, op1=ALU.add)
                    if ell < 3:
                        # shared transpose for q-build and kv-build
                        htp = ps_big.tile([HID, 512], F32, name="htp", space="PSUM")
                        nc.tensor.transpose(htp[:, :128], in_=outsb[:], identity=iden[:])
                        hts = sb_ht.tile([HID, 512], F32, name="hts")
                        nc.scalar.copy(hts[:, :128], htp[:, :128])
                        own_q_build((ell + 1) % 2, t, hts)
                        own_kv_build(ell, t, hts)
                        if t + 1 in qb[1:]:
                            h_ = qb.index(t + 1) - 1
                            fire_ag(ell, h_)
                            copy_ag(ell, h_)
                    else:
                        final_tile(outsb, t)

            def final_tile(outsb, t):
                htp = ps_big.tile([HID, 512], F32, name="htp", space="PSUM")
                nc.tensor.transpose(htp[:, :128], in_=outsb[:], identity=iden[:])
                hts = sb_ht.tile([HID, 512], F32, name="hts")
                nc.scalar.copy(hts[:, :128], htp[:, :128])
                yp = ps_kv.tile([128, 2 * HID], F32, name="kvp", space="PSUM")
                nc.tensor.matmul(yp[:, :OUTC], lhsT=hts[:, :128], rhs=wout_s[:],
                                 start=True, stop=False)
                nc.tensor.matmul(yp[:, :OUTC], lhsT=ones_r[:], rhs=bout_s[:],
                                 start=False, stop=True)
                ysb = sb_out.tile([128, OUTC], F32, name="ysb")
                nc.scalar.copy(ysb[:], yp[:, :OUTC])
                nc.sync.dma_start(y[t * 128:(t + 1) * 128, :], ysb[:])

            # ================= schedule =================
            build_slice0()
            fire_ag(0, 3)
            copy_ag(0, 3)
            edge_layer(1)
            edge_layer(2)
            edge_layer(3)

    nc.compile()
    return nc


def assemble_output(cfg, results):
    n = cfg["n_nodes"]
    out = np.zeros((n, OUTC), np.float32)
    for c in range(cfg["n_cores"]):
        cn = cfg["core_nodes"][c]
        out[cn] = results[c]["y"][:len(cn)]
    return out


# ======================= harness entry point =======================
LAST_EXEC_NS = [None]
LAST_RESULT = [None]


def kernel(**inputs):
    """Full (unsharded) inputs -> full [N, 16] float32 output."""
    from concourse.bass_utils import run_bass_kernel_spmd

    x = np.asarray(inputs["x"], np.float32)
    edge_index = np.asarray(inputs["edge_index"])
    cfg = make_cfg(x.shape[0], edge_index, n_cores=8)
    in_maps = prep_inputs(
        cfg, x,
        inputs["W_lin"], inputs["b_lin"],
        inputs["Wq"], inputs["bq"],
        inputs["Wk"], inputs["bk"],
        inputs["Wv"], inputs["bv"],
        inputs["W_out"], inputs["b_out"],
    )
    nc = build_kernel(cfg)
    res = run_bass_kernel_spmd(nc, in_maps, core_ids=list(range(cfg["n_cores"])))
    LAST_EXEC_NS[0] = res.exec_time_ns
    LAST_RESULT[0] = res
    return assemble_output(cfg, res.results)
